# revision 60
# baseline (speedup 1.0000x reference)
"""Trainium2 Bass kernel for nn_Attention (LN -> QKV proj -> partial RoPE ->
null-KV prepend -> causal MQA attention -> out proj).

Dispatch-cost-aware sharding: the axon PJRT path costs ~10ms fixed +
~0.9ms/core + ~0.8ms/buffer per exec, with payload bytes nearly free.
So: NCORES cores (default 2), each computing NB=4//NCORES full batches
(all 16 heads), with ONE packed f32 input blob + ONE f32 output tensor
per core. Output is a disjoint batch stack (no host reduction).

RoPE is applied post-projection via stream_shuffle partition rotation of
the biased q/k/v rows (rot contribution = shuffle * signed-sin + q * cos),
so no separate rot-weight projections are needed.

All compute ops keep uniform start-partitions (walrus checkSBSameStartPartition):
- k is projected twice (rows 0:64 and 64:128) so odd heads' QK matmuls run with
  lhsT/rhs both at base 64.
- rope groups live at rows base+(0:32) for base in {0, 64}; shuffles and
  combines stay within one base.
"""

import sys

for _p in ("/opt/trn_rl_repo",):
    if _p not in sys.path:
        sys.path.insert(0, _p)

import numpy as np
import ml_dtypes

import concourse.bass as bass
import concourse.tile as tile
from concourse import bacc, mybir
from concourse import bass_utils

F32 = np.float32
BF16 = ml_dtypes.bfloat16

B, N, DIM = 4, 1024, 1024
HEADS, DH = 16, 64
PC = HEADS // 2             # 8 head-pair groups, all on one core
ROT = 32
NN = 2                      # null kv
EPS = 1e-5
P = 128
NEG = -1.0e38
SCALE = DH ** -0.5
NT = N // P                 # 8 i-tiles / D-chunks
IB = N // 512               # 2 i-blocks

NB = 4                      # batches per core
NCORES = B // NB

dt = mybir.dt

ROT_SHUF = list(range(16, 32)) + list(range(0, 16))


def _chunks_for_block(b0):
    """j-tile chunks per i-block: lists of seq j-tile indices; 'T' = tail."""
    if b0 == 0:
        return [[0, 1], [2, 3], ["T"]]
    return [[0, 1], [2, 3], [4, 5], [6, 7], ["T"]]


def _prime_act_tables(arch):
    """Make Exp/Ln resolve to the single set containing both, so the
    act-table insertion pass emits one load instead of thrashing."""
    import concourse.hw_specs as hw_specs
    AF = mybir.ActivationFunctionType
    tables = hw_specs.get_activation_tables(arch)
    if "natural_log_exp_and_others" in tables:
        for name, fns in tables.items():
            if name != "natural_log_exp_and_others":
                for f in (AF.Exp, AF.Ln, AF.Square, AF.Identity, AF.Copy):
                    fns.discard(f)


def _blob_rows(nb, mask_trivial):
    """Row offsets of each section in the packed [R, 1024] f32 blob."""
    off = {}
    r = 0
    off["x"] = r; r += nb * N
    off["wq"] = r; r += DIM          # [1024, 1024]
    off["wkv"] = r; r += DIM         # cols 0:128 = [Wk|Wk], 128:192 = Wv
    off["wout"] = r; r += DIM        # [1024, 1024]
    off["cos"] = r; r += P           # [128, 1024]
    off["sinm"] = r; r += P          # signed sin, [128, 1024]
    off["tri"] = r; r += 3 * P       # [128,2560] as 3 bands (1024,1024,512)
    off["misc"] = r; r += P          # ktail|ident|vtail|qb|kb|vb
    if not mask_trivial:
        off["mb"] = r; r += nb * 4 * P   # per-batch [128, 4096] as 4 bands
    off["_total"] = r
    return off


# misc band column layout
MC_KTAIL = 0          # [128, 128]
MC_IDENT = 128        # [128, 128]
MC_VTAIL = 256        # [128, 66]
MC_QB = 322           # [128, 8] f32
MC_KB = 330           # [128, 1] f32
MC_VB = 331           # [64, 1] f32
MC_TAILB = 332        # [128, 1] f32: 0 rows<NN else NEG (null-tail row mask)


def _build_program(nb, mask_trivial):
    nc = bacc.Bacc("TRN2", target_bir_lowering=False, debug=False)
    _prime_act_tables(nc.m.arch)

    f32, bf16 = dt.float32, dt.bfloat16
    AF = mybir.ActivationFunctionType
    OP = mybir.AluOpType

    R = _blob_rows(nb, mask_trivial)
    d_blob = nc.dram_tensor("blob", [R["_total"], 1024], bf16,
                            kind="ExternalInput")
    d_out = nc.dram_tensor("out", [nb * N, DIM], bf16,
                           kind="ExternalOutput")

    def bap(key, r0, r1, c0, c1):
        return d_blob.ap()[R[key] + r0: R[key] + r1, c0:c1]

    with tile.TileContext(nc) as tc:
        from contextlib import ExitStack

        ctx = ExitStack()
        with ctx:
            consts = ctx.enter_context(tc.tile_pool(name="consts", bufs=1))
            persist = ctx.enter_context(tc.tile_pool(name="persist", bufs=1))

            # ---- persistent SBUF tensors ----
            wq_sb = consts.tile([P, NT * 1024], bf16)      # 8 chunks x [128,1024]
            wkk_sb = consts.tile([P, NT * 128], bf16)
            wv_sb = consts.tile([P, NT * 64], bf16)
            wout_sb = consts.tile([P, PC * DIM], bf16)     # 8 pair chunks
            cos_sb = consts.tile([P, N], bf16)
            sinm_sb = consts.tile([P, N], bf16)
            tri_sb = consts.tile([P, 5 * 512], bf16)
            ktail_sb = consts.tile([P, P], bf16)
            vtail_sb = consts.tile([P, DH + 2], bf16)
            ident = consts.tile([P, P], bf16)
            qb_sb = consts.tile([P, PC], f32)
            kb_sb = consts.tile([P, 1], f32)
            vb_sb = consts.tile([DH, 1], f32)
            tailb_sb = consts.tile([P, 1], f32)
            mb_sb = None
            if not mask_trivial:
                mb_sb = persist.tile([P, NT * 512], bf16)

            qp = persist.tile([P, PC * N], bf16)           # q pairs [128, i]
            kT = persist.tile([P, N], bf16)                # k duplicated rows
            vT = persist.tile([DH, N], bf16)
            vext = persist.tile([P, 9 * (DH + 2)], bf16)   # v + dual ones cols
            ao = persist.tile([P, PC * N], bf16)           # attn out pairs

            # ---- load weights (bf16 blob -> SBUF, direct DMA) ----
            with tc.tile_pool(name="wstg", bufs=2) as stg:
                for c in range(NT):
                    nc.sync.dma_start(wq_sb[:, c * 1024:(c + 1) * 1024],
                                      bap("wq", c * P, (c + 1) * P, 0, 1024))
                    nc.sync.dma_start(wkk_sb[:, c * 128:(c + 1) * 128],
                                      bap("wkv", c * P, (c + 1) * P, 0, 128))
                    nc.sync.dma_start(wv_sb[:, c * 64:(c + 1) * 64],
                                      bap("wkv", c * P, (c + 1) * P, 128, 192))
                for p in range(PC):
                    nc.sync.dma_start(wout_sb[:, p * DIM:(p + 1) * DIM],
                                      bap("wout", p * P, (p + 1) * P, 0, 1024))
                nc.sync.dma_start(cos_sb[:], bap("cos", 0, P, 0, 1024))
                nc.sync.dma_start(sinm_sb[:], bap("sinm", 0, P, 0, 1024))
                nc.sync.dma_start(tri_sb[:, 0:1024], bap("tri", 0, P, 0, 1024))
                nc.sync.dma_start(tri_sb[:, 1024:2048],
                                  bap("tri", P, 2 * P, 0, 1024))
                nc.sync.dma_start(tri_sb[:, 2048:2560],
                                  bap("tri", 2 * P, 3 * P, 0, 512))
                nc.sync.dma_start(ktail_sb[:],
                                  bap("misc", 0, P, MC_KTAIL, MC_KTAIL + P))
                nc.sync.dma_start(ident[:],
                                  bap("misc", 0, P, MC_IDENT, MC_IDENT + P))
                nc.sync.dma_start(vtail_sb[:],
                                  bap("misc", 0, P, MC_VTAIL, MC_VTAIL + DH + 2))
                bst = stg.tile([P, 16], bf16, tag="bst", name="bst")
                nc.sync.dma_start(bst[:, 0:PC],
                                  bap("misc", 0, P, MC_QB, MC_QB + PC))
                nc.sync.dma_start(bst[:, PC:PC + 1],
                                  bap("misc", 0, P, MC_KB, MC_KB + 1))
                nc.sync.dma_start(bst[0:DH, PC + 1:PC + 2],
                                  bap("misc", 0, DH, MC_VB, MC_VB + 1))
                nc.sync.dma_start(bst[:, PC + 2:PC + 3],
                                  bap("misc", 0, P, MC_TAILB, MC_TAILB + 1))
                nc.vector.tensor_copy(qb_sb[:], bst[:, 0:PC])
                nc.vector.tensor_copy(kb_sb[:], bst[:, PC:PC + 1])
                nc.vector.tensor_copy(vb_sb[:], bst[0:DH, PC + 1:PC + 2])
                nc.vector.tensor_copy(tailb_sb[:], bst[:, PC + 2:PC + 3])

            # ---- helpers (same structure as 8-head version, PC=8) ----
            def ln_reduce_tile(ph1, t, xt, rsums, accs):
                c4 = t % 4
                nc.vector.tensor_reduce(rsums[:, c4:c4 + 1], xt[:],
                                        axis=mybir.AxisListType.X, op=OP.add)
                sq = ph1.tile([P, DIM], bf16, tag="sq", name="sq")
                nc.scalar.activation(sq[:], xt[:], AF.Square,
                                     accum_out=accs[:, c4:c4 + 1])

            def ln_stats_batch(stp, rsums, accs):
                mean = stp.tile([P, 4], f32, tag="stb", name="mean")
                nc.vector.tensor_scalar(out=mean[:], in0=rsums[:],
                                        scalar1=1.0 / DIM, scalar2=None,
                                        op0=OP.mult)
                ex2 = stp.tile([P, 4], f32, tag="stb", name="ex2")
                nc.vector.tensor_scalar(out=ex2[:], in0=accs[:],
                                        scalar1=1.0 / DIM, scalar2=None,
                                        op0=OP.mult)
                var = stp.tile([P, 4], f32, tag="stb", name="var")
                nc.vector.scalar_tensor_tensor(
                    out=var[:], in0=mean[:], scalar=-1.0, in1=mean[:],
                    op0=OP.mult, op1=OP.mult)
                nc.vector.scalar_tensor_tensor(
                    out=var[:], in0=ex2[:], scalar=EPS, in1=var[:],
                    op0=OP.add, op1=OP.add)
                nc.scalar.activation(var[:], var[:], AF.Ln)
                rstd = stp.tile([P, 4], f32, tag="stb", name="rstd")
                nc.scalar.activation(rstd[:], var[:], AF.Exp, scale=-0.5)
                negmr = stp.tile([P, 4], f32, tag="stb", name="negmr")
                nc.vector.scalar_tensor_tensor(
                    out=negmr[:], in0=mean[:], scalar=-1.0, in1=rstd[:],
                    op0=OP.mult, op1=OP.mult)
                return rstd, negmr

            def ln_xn_tile(xnT, ph1, ps1, t, xt, rstd, negmr):
                c4 = t % 4
                xn = ph1.tile([P, DIM], bf16, tag="xn", name="xn")
                nc.vector.tensor_scalar(out=xn[:], in0=xt[:],
                                        scalar1=rstd[:, c4:c4 + 1],
                                        scalar2=negmr[:, c4:c4 + 1],
                                        op0=OP.mult, op1=OP.add)
                for g in range(2):
                    pst = ps1.tile([P, 512], bf16, tag="tp", name="pst")
                    for c4b in range(4):
                        c = g * 4 + c4b
                        nc.tensor.transpose(pst[:, c4b * P:(c4b + 1) * P],
                                            xn[:, c * P:(c + 1) * P], ident[:])
                    dest = xnT[:].rearrange("p (c i) -> p c i", c=NT)[
                        :, g * 4:(g + 1) * 4, t * P:(t + 1) * P]
                    src = pst[:].rearrange("p (c i) -> p c i", c=4)
                    nc.scalar.copy(dest, src)

            def mm_proj(xnT, ps2, w_sb, wwidth, col0, cols, ib, rows=P):
                ps = ps2.tile([P, 512], f32, tag="proj", name="ps")
                for c in range(NT):
                    nc.tensor.matmul(
                        ps[0:rows, :],
                        w_sb[:, c * wwidth + col0: c * wwidth + col0 + cols],
                        xnT[:, c * N + ib * 512: c * N + ib * 512 + 512],
                        start=(c == 0), stop=(c == NT - 1))
                return ps

            def rope_rows(rp, dst, base, isl_c, sin_cols):
                """dst rows base:base+32 (cols isl_c slice of width 512):
                dst = dst*cos + shuffle(dst)*sinm."""
                rsl = slice(base, base + ROT)
                tmp = rp.tile([P, 512], bf16, tag="rt", name="rt")
                nc.vector.stream_shuffle(tmp[rsl, :], dst[rsl, isl_c], ROT_SHUF)
                nc.vector.tensor_tensor(out=dst[rsl, isl_c],
                                        in0=dst[rsl, isl_c],
                                        in1=cos_sb[rsl, sin_cols], op=OP.mult)
                nc.vector.tensor_tensor(out=tmp[rsl, :], in0=tmp[rsl, :],
                                        in1=sinm_sb[rsl, sin_cols], op=OP.mult)
                nc.vector.tensor_tensor(out=dst[rsl, isl_c],
                                        in0=dst[rsl, isl_c],
                                        in1=tmp[rsl, :], op=OP.add)

            def proj_ib(xnT, ps2, vtp, rp, ib):
                isl = slice(ib * 512, (ib + 1) * 512)
                for p in range(PC):
                    csl = slice(p * N + ib * 512, p * N + ib * 512 + 512)
                    ps = mm_proj(xnT, ps2, wq_sb, 1024, p * P, P, ib)
                    nc.scalar.add(qp[:, csl], ps[:], qb_sb[:, p:p + 1])
                    for base in (0, DH):
                        rope_rows(rp, qp, base, csl, isl)
                ps = mm_proj(xnT, ps2, wkk_sb, 128, 0, P, ib)
                nc.scalar.add(kT[:, isl], ps[:], kb_sb[:])
                for base in (0, DH):
                    rope_rows(rp, kT, base, isl, isl)
                ps = mm_proj(xnT, ps2, wv_sb, 64, 0, DH, ib, rows=DH)
                nc.scalar.add(vT[:, isl], ps[0:DH, :], vb_sb[:])
                rope_rows(rp, vT, 0, isl, isl)
                # v row-major + dual ones cols for this i-block's j-tiles
                for jj in range(ib * 4, ib * 4 + 4):
                    pv = vtp.tile([P, DH], bf16, tag="vt", name="pv")
                    nc.tensor.transpose(pv[:], vT[:, jj * P:(jj + 1) * P],
                                        ident[0:DH, 0:DH])
                    vbase = jj * (DH + 2)
                    nc.vector.tensor_copy(vext[:, vbase:vbase + DH], pv[:])
                    nc.vector.memset(vext[:, vbase + DH:vbase + DH + 2], 1.0)

            # ================= per-batch pipeline =================
            for b in range(nb):
                if not mask_trivial:
                    for band in range(4):
                        nc.sync.dma_start(
                            mb_sb[:, band * 1024:(band + 1) * 1024],
                            bap("mb", (b * 4 + band) * P,
                                (b * 4 + band + 1) * P, 0, 1024))

                # ---- Phases 1+2: LN + projections + rope ----
                with tc.tile_pool(name="ph1sb", bufs=4) as ph1, \
                     tc.tile_pool(name="ph1st", bufs=32) as stp, \
                     tc.tile_pool(name="xnp", bufs=1) as xnp, \
                     tc.tile_pool(name="ph1ps", bufs=2, space="PSUM") as ps1, \
                     tc.tile_pool(name="ph2ps", bufs=5, space="PSUM") as ps2, \
                     tc.tile_pool(name="rope", bufs=4) as rp, \
                     tc.tile_pool(name="vtp", bufs=1, space="PSUM") as vtp:
                    xnT = xnp.tile([P, NT * N], bf16, tag="xnT", name="xnT")
                    xts = []
                    for t in range(NT):
                        xt = ph1.tile([P, DIM], bf16, tag=f"x{t % 4}",
                                      name=f"xt{t}", bufs=2)
                        nc.gpsimd.dma_start(
                            xt[:], bap("x", b * N + t * P, b * N + (t + 1) * P,
                                       0, 1024))
                        xts.append(xt)
                    for half in range(2):
                        rsums = stp.tile([P, 4], f32, tag=f"rs{half}",
                                         name=f"rsums{half}", bufs=1)
                        accs = stp.tile([P, 4], f32, tag=f"ac{half}",
                                        name=f"accs{half}", bufs=1)
                        for t in range(half * 4, half * 4 + 4):
                            ln_reduce_tile(ph1, t, xts[t], rsums, accs)
                        rstd, negmr = ln_stats_batch(stp, rsums, accs)
                        for t in range(half * 4, half * 4 + 4):
                            ln_xn_tile(xnT, ph1, ps1, t, xts[t], rstd, negmr)
                        proj_ib(xnT, ps2, vtp, rp, half)
                    nc.vector.tensor_copy(vext[:, 8 * (DH + 2):9 * (DH + 2)],
                                          vtail_sb[:])

                # ---- Phase 3: attention (pair-packed) ----
                with tc.tile_pool(name="simps", bufs=3, space="PSUM") as simps, \
                     tc.tile_pool(name="outps", bufs=1, space="PSUM") as outps, \
                     tc.tile_pool(name="atsb", bufs=6) as atsb, \
                     tc.tile_pool(name="nrm", bufs=3) as nrm:
                    for pc in range(PC):
                        rsb = nrm.tile([P, N], f32, name="rsb", tag="rsb")
                        nc.vector.memset(rsb[DH:DH + ROT, :], 1.0)
                        aots = {}
                        for b0 in range(IB):
                            chunks = _chunks_for_block(b0)
                            alljj = [jj for ch in chunks for jj in ch]
                            qhs = {}
                            psos = {}
                            for e in (0, 1):
                                hb = e * DH
                                qhs[e] = qp[hb:hb + DH,
                                            pc * N + b0 * 512:
                                            pc * N + b0 * 512 + 512]
                                psos[e] = outps.tile([P, 512], f32,
                                                     name=f"pso{e}",
                                                     tag=f"outT{e}")
                            first_av = True
                            for ch in chunks:
                                w = len(ch) * 512
                                pss = {}
                                for e in (0, 1):
                                    pss[e] = simps.tile([P, 1024], f32,
                                                        name=f"pss{e}",
                                                        tag="sim")
                                for idx, jj in enumerate(ch):
                                    for e in (0, 1):
                                        hb = e * DH
                                        seg = pss[e][:, idx * 512:(idx + 1) * 512]
                                        diag = jj != "T" and jj >= 4 * b0
                                        if jj == "T":
                                            # tail row-mask rides the exp bias
                                            extra = 0 if mask_trivial else 1
                                        else:
                                            extra = ((1 if diag else 0)
                                                     + (0 if mask_trivial
                                                        else 1))
                                        if jj == "T":
                                            nc.tensor.matmul(
                                                seg, ktail_sb[hb:hb + DH, :],
                                                qhs[e], start=True,
                                                stop=(extra == 0))
                                        else:
                                            nc.tensor.matmul(
                                                seg,
                                                kT[hb:hb + DH,
                                                   jj * P:(jj + 1) * P],
                                                qhs[e], start=True,
                                                stop=(extra == 0))
                                for idx, jj in enumerate(ch):
                                    for e in (0, 1):
                                        seg = pss[e][:, idx * 512:(idx + 1) * 512]
                                        if jj == "T":
                                            if not mask_trivial:
                                                nc.tensor.matmul(
                                                    seg, ident[:],
                                                    tri_sb[:, 4 * 512:5 * 512],
                                                    start=False, stop=True)
                                            continue
                                        diag = jj >= 4 * b0
                                        extra = ((1 if diag else 0)
                                                 + (0 if mask_trivial else 1))
                                        if diag:
                                            k = jj - 4 * b0
                                            extra -= 1
                                            nc.tensor.matmul(
                                                seg, ident[:],
                                                tri_sb[:, k * 512:(k + 1) * 512],
                                                start=False, stop=(extra == 0))
                                        if not mask_trivial:
                                            extra -= 1
                                            nc.tensor.matmul(
                                                seg, ident[:],
                                                mb_sb[:, jj * 512:(jj + 1) * 512],
                                                start=False, stop=(extra == 0))
                                ats = {}
                                for e in (0, 1):
                                    at = atsb.tile([P, 1024], bf16,
                                                   name=f"at{e}", tag=f"at{e}")
                                    if mask_trivial and ch == ["T"]:
                                        nc.scalar.activation(at[:, 0:w],
                                                             pss[e][:, 0:w],
                                                             AF.Exp, scale=SCALE,
                                                             bias=tailb_sb[:])
                                    else:
                                        nc.scalar.activation(at[:, 0:w],
                                                             pss[e][:, 0:w],
                                                             AF.Exp, scale=SCALE)
                                    ats[e] = at
                                for idx, jj in enumerate(ch):
                                    vjj = 8 if jj == "T" else jj
                                    vcols = vext[:, vjj * (DH + 2):
                                                 (vjj + 1) * (DH + 2)]
                                    for e in (0, 1):
                                        nc.tensor.matmul(
                                            psos[e][0:DH + 2, :], vcols,
                                            ats[e][:, idx * 512:(idx + 1) * 512],
                                            start=first_av,
                                            stop=(jj == alljj[-1]))
                                    first_av = False
                            bsl0 = slice(b0 * 512, (b0 + 1) * 512)
                            for e in (1, 0):
                                aot = nrm.tile([DH + 2, 512], f32,
                                               name=f"aot{b0}{e}",
                                               tag=f"aot{b0}{e}")
                                nc.vector.tensor_copy(aot[:],
                                                      psos[e][0:DH + 2, :])
                                if e == 1:
                                    nc.vector.tensor_copy(rsb[DH:DH + 2, bsl0],
                                                          aot[DH:DH + 2, :])
                                else:
                                    nc.vector.tensor_copy(rsb[DH:DH + 1, bsl0],
                                                          aot[DH:DH + 1, :])
                                aots[(b0, e)] = aot
                        rows2 = rsb[DH:DH + 2, :]
                        nc.scalar.activation(rows2, rows2, AF.Ln)
                        nc.scalar.activation(rows2, rows2, AF.Exp, scale=-1.0)
                        for e in (0, 1):
                            bc = nrm.tile([P, N], f32, name=f"bc{e}",
                                          tag=f"bc{e}")
                            nc.vector.stream_shuffle(bc[DH:DH + ROT, :],
                                                     rsb[DH:DH + ROT, :],
                                                     [e] * 32)
                            nc.sync.dma_start(bc[0:ROT, :], bc[DH:DH + ROT, :])
                            nc.sync.dma_start(bc[ROT:DH, :], bc[0:ROT, :])
                            for b0 in range(IB):
                                osl = slice(pc * N + b0 * 512,
                                            pc * N + b0 * 512 + 512)
                                bsl = slice(b0 * 512, (b0 + 1) * 512)
                                src = aots[(b0, e)]
                                if e == 0:
                                    nc.gpsimd.tensor_tensor(
                                        out=ao[0:DH, osl], in0=src[0:DH, :],
                                        in1=bc[0:DH, bsl], op=OP.mult)
                                else:
                                    tmp = nrm.tile([DH, 512], bf16,
                                                   name="tmpn", tag="tmpn")
                                    nc.gpsimd.tensor_tensor(
                                        out=tmp[:], in0=src[0:DH, :],
                                        in1=bc[0:DH, bsl], op=OP.mult)
                                    nc.sync.dma_start(ao[DH:P, osl], tmp[:])

                # ---- Phase 4: out projection ----
                with tc.tile_pool(name="opps", bufs=4, space="PSUM") as opps, \
                     tc.tile_pool(name="opsb", bufs=3) as opsb:
                    for t in range(NT):
                        orow = opsb.tile([P, DIM], bf16, tag="orow")
                        for nb2 in range(2):
                            ps = opps.tile([P, 512], f32, tag="op")
                            for p in range(PC):
                                nc.tensor.matmul(
                                    ps[:],
                                    ao[:, p * N + t * P: p * N + t * P + 128],
                                    wout_sb[:, p * DIM + nb2 * 512:
                                            p * DIM + nb2 * 512 + 512],
                                    start=(p == 0), stop=(p == PC - 1))
                            nc.scalar.copy(
                                orow[:, nb2 * 512:(nb2 + 1) * 512], ps[:])
                        nc.sync.dma_start(
                            d_out.ap()[b * N + t * P: b * N + (t + 1) * P, :],
                            orow[:])

    nc.compile()
    return nc


_PROG_CACHE = {}


def _get_program(mask_trivial, nb=NB):
    key = (nb, bool(mask_trivial))
    if key not in _PROG_CACHE:
        _PROG_CACHE[key] = _build_program(nb, key[1])
    return _PROG_CACHE[key]


def _host_prep(core, x, mask, freqs, ln_g, ln_b, W_q, W_kv, W_out, null_kv,
               mask_trivial, nb=NB):
    R = _blob_rows(nb, mask_trivial)
    blob = np.zeros((R["_total"], 1024), BF16)

    for i in range(nb):
        blob[R["x"] + i * N: R["x"] + (i + 1) * N, :] = x[core * nb + i]

    Wq_eff = W_q * ln_g[:, None]                        # [1024, 1024]
    Wkv_eff = W_kv * ln_g[:, None]                      # [1024, 128]
    bq = ln_b @ W_q                                     # [1024]
    bkv = ln_b @ W_kv                                   # [128]
    Wk, Wv = Wkv_eff[:, 0:DH], Wkv_eff[:, DH:2 * DH]
    bk, bv = bkv[0:DH], bkv[DH:2 * DH]

    blob[R["wq"]:R["wq"] + DIM, :] = Wq_eff
    blob[R["wkv"]:R["wkv"] + DIM, 0:DH] = Wk
    blob[R["wkv"]:R["wkv"] + DIM, DH:2 * DH] = Wk
    blob[R["wkv"]:R["wkv"] + DIM, 128:192] = Wv
    blob[R["wout"]:R["wout"] + DIM, :] = W_out

    f = np.asarray(freqs, np.float64)                   # [1024, 32]
    blob[R["cos"]:R["cos"] + P, :] = np.tile(np.cos(f).T, (4, 1))
    s = np.sin(f).T                                     # [32, 1024]
    sm = s.copy()
    sm[0:ROT // 2, :] = -s[0:ROT // 2, :]
    blob[R["sinm"]:R["sinm"] + P, :] = np.tile(sm, (4, 1))

    tri = np.zeros((P, 5 * 512), F32)
    pidx = np.arange(P)[:, None]
    il = np.arange(512)[None, :]
    for k in range(4):
        tri[:, k * 512:(k + 1) * 512] = np.where(il >= 128 * k + pidx,
                                                 0.0, NEG)
    tri[NN:, 4 * 512:5 * 512] = NEG
    blob[R["tri"]:R["tri"] + P, :] = tri[:, 0:1024]
    blob[R["tri"] + P:R["tri"] + 2 * P, :] = tri[:, 1024:2048]
    blob[R["tri"] + 2 * P:R["tri"] + 3 * P, 0:512] = tri[:, 2048:2560]

    nk = np.asarray(null_kv[0]).T                       # [64, 2]
    blob[R["misc"]:R["misc"] + DH, MC_KTAIL:MC_KTAIL + NN] = nk
    blob[R["misc"] + DH:R["misc"] + P, MC_KTAIL:MC_KTAIL + NN] = nk
    blob[R["misc"]:R["misc"] + P,
         MC_IDENT:MC_IDENT + P] = np.eye(P, dtype=F32)
    blob[R["misc"]:R["misc"] + NN, MC_VTAIL:MC_VTAIL + DH] = \
        np.asarray(null_kv[1])
    blob[R["misc"]:R["misc"] + NN, MC_VTAIL + DH:MC_VTAIL + DH + NN] = 1.0
    for p in range(PC):
        blob[R["misc"]:R["misc"] + P, MC_QB + p] = bq[p * 128:(p + 1) * 128]
    blob[R["misc"]:R["misc"] + P, MC_KB] = np.concatenate([bk, bk])
    blob[R["misc"]:R["misc"] + DH, MC_VB] = bv
    blob[R["misc"] + NN:R["misc"] + P, MC_TAILB] = NEG

    if not mask_trivial:
        for i in range(nb):
            mrow = np.where(np.asarray(mask[core * nb + i]), 0.0, NEG)
            mb = np.zeros((P, NT * 512), F32)
            for jj in range(NT):
                mb[:, jj * 512:(jj + 1) * 512] = \
                    mrow[jj * P:(jj + 1) * P][:, None]
            for band in range(4):
                blob[R["mb"] + (i * 4 + band) * P:
                     R["mb"] + (i * 4 + band + 1) * P, :] = \
                    mb[:, band * 1024:(band + 1) * 1024]

    return {"blob": blob}


def _run(x, mask, freqs, ln_g, ln_b, W_q, W_kv, W_out, null_kv, **spmd_kwargs):
    x = np.asarray(x, F32)
    mask = np.asarray(mask)
    freqs = np.asarray(freqs, F32)
    ln_g = np.asarray(ln_g, np.float64)
    ln_b = np.asarray(ln_b, np.float64)
    W_q = np.asarray(W_q, np.float64)
    W_kv = np.asarray(W_kv, np.float64)
    W_out = np.asarray(W_out, np.float64)
    null_kv = np.asarray(null_kv, F32)

    mask_trivial = bool(mask.all())
    nc = _get_program(mask_trivial)
    in_maps = [
        _host_prep(c, x, mask, freqs, ln_g, ln_b, W_q, W_kv, W_out, null_kv,
                   mask_trivial)
        for c in range(NCORES)
    ]
    res = bass_utils.run_bass_kernel_spmd(nc, in_maps, list(range(NCORES)),
                                          **spmd_kwargs)
    out = np.empty((B, N, DIM), F32)
    for b in range(B):
        out[b] = res.results[b // NB]["out"][(b % NB) * N:(b % NB + 1) * N]
    return out, res


def kernel(x, mask, freqs, ln_g, ln_b, W_q, W_kv, W_out, null_kv):
    out, _ = _run(x, mask, freqs, ln_g, ln_b, W_q, W_kv, W_out, null_kv)
    return out


# revision 62
# speedup vs baseline: 1.1412x; 1.1412x over previous
"""Trainium2 Bass kernel for nn_Attention (LN -> QKV proj -> partial RoPE ->
null-KV prepend -> causal MQA attention -> out proj).

Dispatch-cost-aware sharding: the axon PJRT path has ~80ms pipeline-fill
latency per burst and ~1.3-1.6ms/call steady-state, growing with cores
used and buffer count while payload bytes are nearly free. So: ONE core
computing all NB=4 batches (all 16 heads), with ONE packed bf16 input
blob + ONE bf16 output tensor. Output is a disjoint batch stack (no
host reduction). Null-tail row masking rides the softmax Exp's bias AP;
causal tri masking uses K=128 identity matmuls into sim PSUM.

RoPE is applied post-projection via stream_shuffle partition rotation of
the biased q/k/v rows (rot contribution = shuffle * signed-sin + q * cos),
so no separate rot-weight projections are needed.

All compute ops keep uniform start-partitions (walrus checkSBSameStartPartition):
- k is projected twice (rows 0:64 and 64:128) so odd heads' QK matmuls run with
  lhsT/rhs both at base 64.
- rope groups live at rows base+(0:32) for base in {0, 64}; shuffles and
  combines stay within one base.
"""

import sys

for _p in ("/opt/trn_rl_repo",):
    if _p not in sys.path:
        sys.path.insert(0, _p)

import numpy as np
import ml_dtypes

import concourse.bass as bass
import concourse.tile as tile
from concourse import bacc, mybir
from concourse import bass_utils

F32 = np.float32
BF16 = ml_dtypes.bfloat16

B, N, DIM = 4, 1024, 1024
HEADS, DH = 16, 64
PC = HEADS // 2             # 8 head-pair groups, all on one core
ROT = 32
NN = 2                      # null kv
EPS = 1e-5
P = 128
NEG = -1.0e38
SCALE = DH ** -0.5
NT = N // P                 # 8 i-tiles / D-chunks
IB = N // 512               # 2 i-blocks

NB = 4                      # batches per core
NCORES = B // NB

dt = mybir.dt

ROT_SHUF = list(range(16, 32)) + list(range(0, 16))


def _chunks_for_block(b0):
    """j-tile chunks per i-block: lists of seq j-tile indices; 'T' = tail."""
    if b0 == 0:
        return [[0, 1], [2, 3], ["T"]]
    return [[0, 1], [2, 3], [4, 5], [6, 7], ["T"]]


def _prime_act_tables(arch):
    """Make Exp/Ln resolve to the single set containing both, so the
    act-table insertion pass emits one load instead of thrashing."""
    import concourse.hw_specs as hw_specs
    AF = mybir.ActivationFunctionType
    tables = hw_specs.get_activation_tables(arch)
    if "natural_log_exp_and_others" in tables:
        for name, fns in tables.items():
            if name != "natural_log_exp_and_others":
                for f in (AF.Exp, AF.Ln, AF.Square, AF.Identity, AF.Copy):
                    fns.discard(f)


def _blob_rows(nb, mask_trivial):
    """Row offsets of each section in the packed [R, 1024] bf16 blob."""
    off = {}
    r = 0
    off["x"] = r; r += nb * N
    off["wq"] = r; r += DIM          # [1024, 1024]
    off["wkv"] = r; r += DIM         # cols 0:128 = [Wk|Wk], 128:192 = Wv
    off["wout"] = r; r += DIM        # [1024, 1024]
    off["cos"] = r; r += P           # [128, 1024]
    off["sinm"] = r; r += P          # signed sin, [128, 1024]
    off["tri"] = r; r += 3 * P       # [128,2560] as 3 bands (1024,1024,512)
    off["misc"] = r; r += P          # ktail|ident|vtail|qb|kb|vb
    if not mask_trivial:
        off["mb"] = r; r += nb * 4 * P   # per-batch [128, 4096] as 4 bands
    off["_total"] = r
    return off


# misc band column layout
MC_KTAIL = 0          # [128, 128]
MC_IDENT = 128        # [128, 128]
MC_VTAIL = 256        # [128, 66]
MC_QB = 322           # [128, 8] f32
MC_KB = 330           # [128, 1] f32
MC_VB = 331           # [64, 1] f32
MC_TAILB = 332        # [128, 1] f32: 0 rows<NN else NEG (null-tail row mask)


def _build_program(nb, mask_trivial):
    nc = bacc.Bacc("TRN2", target_bir_lowering=False, debug=False)
    _prime_act_tables(nc.m.arch)

    f32, bf16 = dt.float32, dt.bfloat16
    AF = mybir.ActivationFunctionType
    OP = mybir.AluOpType

    R = _blob_rows(nb, mask_trivial)
    d_blob = nc.dram_tensor("blob", [R["_total"], 1024], bf16,
                            kind="ExternalInput")
    d_out = nc.dram_tensor("out", [nb * N, DIM], bf16,
                           kind="ExternalOutput")

    def bap(key, r0, r1, c0, c1):
        return d_blob.ap()[R[key] + r0: R[key] + r1, c0:c1]

    with tile.TileContext(nc) as tc:
        from contextlib import ExitStack

        ctx = ExitStack()
        with ctx:
            consts = ctx.enter_context(tc.tile_pool(name="consts", bufs=1))
            persist = ctx.enter_context(tc.tile_pool(name="persist", bufs=1))

            # ---- persistent SBUF tensors ----
            wq_sb = consts.tile([P, NT * 1024], bf16)      # 8 chunks x [128,1024]
            wkk_sb = consts.tile([P, NT * 128], bf16)
            wv_sb = consts.tile([P, NT * 64], bf16)
            wout_sb = consts.tile([P, PC * DIM], bf16)     # 8 pair chunks
            cos_sb = consts.tile([P, N], bf16)
            sinm_sb = consts.tile([P, N], bf16)
            tri_sb = consts.tile([P, 5 * 512], bf16)
            ktail_sb = consts.tile([P, P], bf16)
            vtail_sb = consts.tile([P, DH + 2], bf16)
            ident = consts.tile([P, P], bf16)
            qb_sb = consts.tile([P, PC], f32)
            kb_sb = consts.tile([P, 1], f32)
            vb_sb = consts.tile([DH, 1], f32)
            tailb_sb = consts.tile([P, 1], f32)
            mb_sb = None
            if not mask_trivial:
                mb_sb = persist.tile([P, NT * 512], bf16)

            qp = persist.tile([P, PC * N], bf16)           # q pairs [128, i]
            kT = persist.tile([P, N], bf16)                # k duplicated rows
            vT = persist.tile([DH, N], bf16)
            vext = persist.tile([P, 9 * (DH + 2)], bf16)   # v + dual ones cols
            ao = persist.tile([P, PC * N], bf16)           # attn out pairs

            # ---- load weights (bf16 blob -> SBUF, direct DMA) ----
            with tc.tile_pool(name="wstg", bufs=2) as stg:
                for c in range(NT):
                    nc.sync.dma_start(wq_sb[:, c * 1024:(c + 1) * 1024],
                                      bap("wq", c * P, (c + 1) * P, 0, 1024))
                    nc.sync.dma_start(wkk_sb[:, c * 128:(c + 1) * 128],
                                      bap("wkv", c * P, (c + 1) * P, 0, 128))
                    nc.sync.dma_start(wv_sb[:, c * 64:(c + 1) * 64],
                                      bap("wkv", c * P, (c + 1) * P, 128, 192))
                for p in range(PC):
                    nc.sync.dma_start(wout_sb[:, p * DIM:(p + 1) * DIM],
                                      bap("wout", p * P, (p + 1) * P, 0, 1024))
                nc.sync.dma_start(cos_sb[:], bap("cos", 0, P, 0, 1024))
                nc.sync.dma_start(sinm_sb[:], bap("sinm", 0, P, 0, 1024))
                nc.sync.dma_start(tri_sb[:, 0:1024], bap("tri", 0, P, 0, 1024))
                nc.sync.dma_start(tri_sb[:, 1024:2048],
                                  bap("tri", P, 2 * P, 0, 1024))
                nc.sync.dma_start(tri_sb[:, 2048:2560],
                                  bap("tri", 2 * P, 3 * P, 0, 512))
                nc.sync.dma_start(ktail_sb[:],
                                  bap("misc", 0, P, MC_KTAIL, MC_KTAIL + P))
                nc.sync.dma_start(ident[:],
                                  bap("misc", 0, P, MC_IDENT, MC_IDENT + P))
                nc.sync.dma_start(vtail_sb[:],
                                  bap("misc", 0, P, MC_VTAIL, MC_VTAIL + DH + 2))
                bst = stg.tile([P, 16], bf16, tag="bst", name="bst")
                nc.sync.dma_start(bst[:, 0:PC],
                                  bap("misc", 0, P, MC_QB, MC_QB + PC))
                nc.sync.dma_start(bst[:, PC:PC + 1],
                                  bap("misc", 0, P, MC_KB, MC_KB + 1))
                nc.sync.dma_start(bst[0:DH, PC + 1:PC + 2],
                                  bap("misc", 0, DH, MC_VB, MC_VB + 1))
                nc.sync.dma_start(bst[:, PC + 2:PC + 3],
                                  bap("misc", 0, P, MC_TAILB, MC_TAILB + 1))
                nc.vector.tensor_copy(qb_sb[:], bst[:, 0:PC])
                nc.vector.tensor_copy(kb_sb[:], bst[:, PC:PC + 1])
                nc.vector.tensor_copy(vb_sb[:], bst[0:DH, PC + 1:PC + 2])
                nc.vector.tensor_copy(tailb_sb[:], bst[:, PC + 2:PC + 3])

            # ---- helpers (same structure as 8-head version, PC=8) ----
            def ln_reduce_tile(ph1, t, xt, rsums, accs):
                c4 = t % 4
                nc.vector.tensor_reduce(rsums[:, c4:c4 + 1], xt[:],
                                        axis=mybir.AxisListType.X, op=OP.add)
                sq = ph1.tile([P, DIM], bf16, tag="sq", name="sq")
                nc.scalar.activation(sq[:], xt[:], AF.Square,
                                     accum_out=accs[:, c4:c4 + 1])

            def ln_stats_batch(stp, rsums, accs):
                mean = stp.tile([P, 4], f32, tag="stb", name="mean")
                nc.vector.tensor_scalar(out=mean[:], in0=rsums[:],
                                        scalar1=1.0 / DIM, scalar2=None,
                                        op0=OP.mult)
                ex2 = stp.tile([P, 4], f32, tag="stb", name="ex2")
                nc.vector.tensor_scalar(out=ex2[:], in0=accs[:],
                                        scalar1=1.0 / DIM, scalar2=None,
                                        op0=OP.mult)
                var = stp.tile([P, 4], f32, tag="stb", name="var")
                nc.vector.scalar_tensor_tensor(
                    out=var[:], in0=mean[:], scalar=-1.0, in1=mean[:],
                    op0=OP.mult, op1=OP.mult)
                nc.vector.scalar_tensor_tensor(
                    out=var[:], in0=ex2[:], scalar=EPS, in1=var[:],
                    op0=OP.add, op1=OP.add)
                nc.scalar.activation(var[:], var[:], AF.Ln)
                rstd = stp.tile([P, 4], f32, tag="stb", name="rstd")
                nc.scalar.activation(rstd[:], var[:], AF.Exp, scale=-0.5)
                negmr = stp.tile([P, 4], f32, tag="stb", name="negmr")
                nc.vector.scalar_tensor_tensor(
                    out=negmr[:], in0=mean[:], scalar=-1.0, in1=rstd[:],
                    op0=OP.mult, op1=OP.mult)
                return rstd, negmr

            def ln_xn_tile(xnT, ph1, ps1, t, xt, rstd, negmr):
                c4 = t % 4
                xn = ph1.tile([P, DIM], bf16, tag="xn", name="xn")
                nc.vector.tensor_scalar(out=xn[:], in0=xt[:],
                                        scalar1=rstd[:, c4:c4 + 1],
                                        scalar2=negmr[:, c4:c4 + 1],
                                        op0=OP.mult, op1=OP.add)
                for g in range(2):
                    pst = ps1.tile([P, 512], bf16, tag="tp", name="pst")
                    for c4b in range(4):
                        c = g * 4 + c4b
                        nc.tensor.transpose(pst[:, c4b * P:(c4b + 1) * P],
                                            xn[:, c * P:(c + 1) * P], ident[:])
                    dest = xnT[:].rearrange("p (c i) -> p c i", c=NT)[
                        :, g * 4:(g + 1) * 4, t * P:(t + 1) * P]
                    src = pst[:].rearrange("p (c i) -> p c i", c=4)
                    nc.scalar.copy(dest, src)

            def mm_proj(xnT, ps2, w_sb, wwidth, col0, cols, ib, rows=P):
                ps = ps2.tile([P, 512], f32, tag="proj", name="ps")
                for c in range(NT):
                    nc.tensor.matmul(
                        ps[0:rows, :],
                        w_sb[:, c * wwidth + col0: c * wwidth + col0 + cols],
                        xnT[:, c * N + ib * 512: c * N + ib * 512 + 512],
                        start=(c == 0), stop=(c == NT - 1))
                return ps

            def rope_rows(rp, dst, base, isl_c, sin_cols):
                """dst rows base:base+32 (cols isl_c slice of width 512):
                dst = dst*cos + shuffle(dst)*sinm."""
                rsl = slice(base, base + ROT)
                tmp = rp.tile([P, 512], bf16, tag="rt", name="rt")
                nc.vector.stream_shuffle(tmp[rsl, :], dst[rsl, isl_c], ROT_SHUF)
                nc.vector.tensor_tensor(out=dst[rsl, isl_c],
                                        in0=dst[rsl, isl_c],
                                        in1=cos_sb[rsl, sin_cols], op=OP.mult)
                nc.vector.tensor_tensor(out=tmp[rsl, :], in0=tmp[rsl, :],
                                        in1=sinm_sb[rsl, sin_cols], op=OP.mult)
                nc.vector.tensor_tensor(out=dst[rsl, isl_c],
                                        in0=dst[rsl, isl_c],
                                        in1=tmp[rsl, :], op=OP.add)

            def proj_ib(xnT, ps2, vtp, rp, ib):
                isl = slice(ib * 512, (ib + 1) * 512)
                for p in range(PC):
                    csl = slice(p * N + ib * 512, p * N + ib * 512 + 512)
                    ps = mm_proj(xnT, ps2, wq_sb, 1024, p * P, P, ib)
                    nc.scalar.add(qp[:, csl], ps[:], qb_sb[:, p:p + 1])
                    for base in (0, DH):
                        rope_rows(rp, qp, base, csl, isl)
                ps = mm_proj(xnT, ps2, wkk_sb, 128, 0, P, ib)
                nc.scalar.add(kT[:, isl], ps[:], kb_sb[:])
                for base in (0, DH):
                    rope_rows(rp, kT, base, isl, isl)
                ps = mm_proj(xnT, ps2, wv_sb, 64, 0, DH, ib, rows=DH)
                nc.scalar.add(vT[:, isl], ps[0:DH, :], vb_sb[:])
                rope_rows(rp, vT, 0, isl, isl)
                # v row-major + dual ones cols for this i-block's j-tiles
                for jj in range(ib * 4, ib * 4 + 4):
                    pv = vtp.tile([P, DH], bf16, tag="vt", name="pv")
                    nc.tensor.transpose(pv[:], vT[:, jj * P:(jj + 1) * P],
                                        ident[0:DH, 0:DH])
                    vbase = jj * (DH + 2)
                    nc.vector.tensor_copy(vext[:, vbase:vbase + DH], pv[:])
                    nc.vector.memset(vext[:, vbase + DH:vbase + DH + 2], 1.0)

            # ================= per-batch pipeline =================
            for b in range(nb):
                if not mask_trivial:
                    for band in range(4):
                        nc.sync.dma_start(
                            mb_sb[:, band * 1024:(band + 1) * 1024],
                            bap("mb", (b * 4 + band) * P,
                                (b * 4 + band + 1) * P, 0, 1024))

                # ---- Phases 1+2: LN + projections + rope ----
                with tc.tile_pool(name="ph1sb", bufs=4) as ph1, \
                     tc.tile_pool(name="ph1st", bufs=32) as stp, \
                     tc.tile_pool(name="xnp", bufs=1) as xnp, \
                     tc.tile_pool(name="ph1ps", bufs=2, space="PSUM") as ps1, \
                     tc.tile_pool(name="ph2ps", bufs=5, space="PSUM") as ps2, \
                     tc.tile_pool(name="rope", bufs=4) as rp, \
                     tc.tile_pool(name="vtp", bufs=1, space="PSUM") as vtp:
                    xnT = xnp.tile([P, NT * N], bf16, tag="xnT", name="xnT")
                    xts = []
                    for t in range(NT):
                        xt = ph1.tile([P, DIM], bf16, tag=f"x{t % 4}",
                                      name=f"xt{t}", bufs=2)
                        nc.gpsimd.dma_start(
                            xt[:], bap("x", b * N + t * P, b * N + (t + 1) * P,
                                       0, 1024))
                        xts.append(xt)
                    for half in range(2):
                        rsums = stp.tile([P, 4], f32, tag=f"rs{half}",
                                         name=f"rsums{half}", bufs=1)
                        accs = stp.tile([P, 4], f32, tag=f"ac{half}",
                                        name=f"accs{half}", bufs=1)
                        for t in range(half * 4, half * 4 + 4):
                            ln_reduce_tile(ph1, t, xts[t], rsums, accs)
                        rstd, negmr = ln_stats_batch(stp, rsums, accs)
                        for t in range(half * 4, half * 4 + 4):
                            ln_xn_tile(xnT, ph1, ps1, t, xts[t], rstd, negmr)
                        proj_ib(xnT, ps2, vtp, rp, half)
                    nc.vector.tensor_copy(vext[:, 8 * (DH + 2):9 * (DH + 2)],
                                          vtail_sb[:])

                # ---- Phase 3: attention (pair-packed) ----
                with tc.tile_pool(name="simps", bufs=3, space="PSUM") as simps, \
                     tc.tile_pool(name="outps", bufs=1, space="PSUM") as outps, \
                     tc.tile_pool(name="atsb", bufs=6) as atsb, \
                     tc.tile_pool(name="nrm", bufs=3) as nrm:
                    for pc in range(PC):
                        rsb = nrm.tile([P, N], f32, name="rsb", tag="rsb")
                        nc.vector.memset(rsb[DH:DH + ROT, :], 1.0)
                        aots = {}
                        for b0 in range(IB):
                            chunks = _chunks_for_block(b0)
                            alljj = [jj for ch in chunks for jj in ch]
                            qhs = {}
                            psos = {}
                            for e in (0, 1):
                                hb = e * DH
                                qhs[e] = qp[hb:hb + DH,
                                            pc * N + b0 * 512:
                                            pc * N + b0 * 512 + 512]
                                psos[e] = outps.tile([P, 512], f32,
                                                     name=f"pso{e}",
                                                     tag=f"outT{e}")
                            first_av = True
                            for ch in chunks:
                                w = len(ch) * 512
                                pss = {}
                                for e in (0, 1):
                                    pss[e] = simps.tile([P, 1024], f32,
                                                        name=f"pss{e}",
                                                        tag="sim")
                                for idx, jj in enumerate(ch):
                                    for e in (0, 1):
                                        hb = e * DH
                                        seg = pss[e][:, idx * 512:(idx + 1) * 512]
                                        diag = jj != "T" and jj >= 4 * b0
                                        if jj == "T":
                                            # tail row-mask rides the exp bias
                                            extra = 0 if mask_trivial else 1
                                        else:
                                            extra = ((1 if diag else 0)
                                                     + (0 if mask_trivial
                                                        else 1))
                                        if jj == "T":
                                            nc.tensor.matmul(
                                                seg, ktail_sb[hb:hb + DH, :],
                                                qhs[e], start=True,
                                                stop=(extra == 0))
                                        else:
                                            nc.tensor.matmul(
                                                seg,
                                                kT[hb:hb + DH,
                                                   jj * P:(jj + 1) * P],
                                                qhs[e], start=True,
                                                stop=(extra == 0))
                                for idx, jj in enumerate(ch):
                                    for e in (0, 1):
                                        seg = pss[e][:, idx * 512:(idx + 1) * 512]
                                        if jj == "T":
                                            if not mask_trivial:
                                                nc.tensor.matmul(
                                                    seg, ident[:],
                                                    tri_sb[:, 4 * 512:5 * 512],
                                                    start=False, stop=True)
                                            continue
                                        diag = jj >= 4 * b0
                                        extra = ((1 if diag else 0)
                                                 + (0 if mask_trivial else 1))
                                        if diag:
                                            k = jj - 4 * b0
                                            extra -= 1
                                            nc.tensor.matmul(
                                                seg, ident[:],
                                                tri_sb[:, k * 512:(k + 1) * 512],
                                                start=False, stop=(extra == 0))
                                        if not mask_trivial:
                                            extra -= 1
                                            nc.tensor.matmul(
                                                seg, ident[:],
                                                mb_sb[:, jj * 512:(jj + 1) * 512],
                                                start=False, stop=(extra == 0))
                                ats = {}
                                for e in (0, 1):
                                    at = atsb.tile([P, 1024], bf16,
                                                   name=f"at{e}", tag=f"at{e}")
                                    if mask_trivial and ch == ["T"]:
                                        nc.scalar.activation(at[:, 0:w],
                                                             pss[e][:, 0:w],
                                                             AF.Exp, scale=SCALE,
                                                             bias=tailb_sb[:])
                                    else:
                                        nc.scalar.activation(at[:, 0:w],
                                                             pss[e][:, 0:w],
                                                             AF.Exp, scale=SCALE)
                                    ats[e] = at
                                for idx, jj in enumerate(ch):
                                    vjj = 8 if jj == "T" else jj
                                    vcols = vext[:, vjj * (DH + 2):
                                                 (vjj + 1) * (DH + 2)]
                                    for e in (0, 1):
                                        nc.tensor.matmul(
                                            psos[e][0:DH + 2, :], vcols,
                                            ats[e][:, idx * 512:(idx + 1) * 512],
                                            start=first_av,
                                            stop=(jj == alljj[-1]))
                                    first_av = False
                            bsl0 = slice(b0 * 512, (b0 + 1) * 512)
                            for e in (1, 0):
                                aot = nrm.tile([DH + 2, 512], f32,
                                               name=f"aot{b0}{e}",
                                               tag=f"aot{b0}{e}")
                                nc.vector.tensor_copy(aot[:],
                                                      psos[e][0:DH + 2, :])
                                if e == 1:
                                    nc.vector.tensor_copy(rsb[DH:DH + 2, bsl0],
                                                          aot[DH:DH + 2, :])
                                else:
                                    nc.vector.tensor_copy(rsb[DH:DH + 1, bsl0],
                                                          aot[DH:DH + 1, :])
                                aots[(b0, e)] = aot
                        rows2 = rsb[DH:DH + 2, :]
                        nc.scalar.activation(rows2, rows2, AF.Ln)
                        nc.scalar.activation(rows2, rows2, AF.Exp, scale=-1.0)
                        for e in (0, 1):
                            bc = nrm.tile([P, N], f32, name=f"bc{e}",
                                          tag=f"bc{e}")
                            nc.vector.stream_shuffle(bc[DH:DH + ROT, :],
                                                     rsb[DH:DH + ROT, :],
                                                     [e] * 32)
                            nc.sync.dma_start(bc[0:ROT, :], bc[DH:DH + ROT, :])
                            nc.sync.dma_start(bc[ROT:DH, :], bc[0:ROT, :])
                            for b0 in range(IB):
                                osl = slice(pc * N + b0 * 512,
                                            pc * N + b0 * 512 + 512)
                                bsl = slice(b0 * 512, (b0 + 1) * 512)
                                src = aots[(b0, e)]
                                if e == 0:
                                    nc.gpsimd.tensor_tensor(
                                        out=ao[0:DH, osl], in0=src[0:DH, :],
                                        in1=bc[0:DH, bsl], op=OP.mult)
                                else:
                                    tmp = nrm.tile([DH, 512], bf16,
                                                   name="tmpn", tag="tmpn")
                                    nc.gpsimd.tensor_tensor(
                                        out=tmp[:], in0=src[0:DH, :],
                                        in1=bc[0:DH, bsl], op=OP.mult)
                                    nc.sync.dma_start(ao[DH:P, osl], tmp[:])

                # ---- Phase 4: out projection ----
                with tc.tile_pool(name="opps", bufs=4, space="PSUM") as opps, \
                     tc.tile_pool(name="opsb", bufs=3) as opsb:
                    for t in range(NT):
                        orow = opsb.tile([P, DIM], bf16, tag="orow")
                        for nb2 in range(2):
                            ps = opps.tile([P, 512], f32, tag="op")
                            for p in range(PC):
                                nc.tensor.matmul(
                                    ps[:],
                                    ao[:, p * N + t * P: p * N + t * P + 128],
                                    wout_sb[:, p * DIM + nb2 * 512:
                                            p * DIM + nb2 * 512 + 512],
                                    start=(p == 0), stop=(p == PC - 1))
                            nc.scalar.copy(
                                orow[:, nb2 * 512:(nb2 + 1) * 512], ps[:])
                        nc.sync.dma_start(
                            d_out.ap()[b * N + t * P: b * N + (t + 1) * P, :],
                            orow[:])

    nc.compile()
    return nc


_PROG_CACHE = {}


def _get_program(mask_trivial, nb=NB):
    key = (nb, bool(mask_trivial))
    if key not in _PROG_CACHE:
        _PROG_CACHE[key] = _build_program(nb, key[1])
    return _PROG_CACHE[key]


def _host_prep(core, x, mask, freqs, ln_g, ln_b, W_q, W_kv, W_out, null_kv,
               mask_trivial, nb=NB):
    R = _blob_rows(nb, mask_trivial)
    blob = np.zeros((R["_total"], 1024), BF16)

    for i in range(nb):
        blob[R["x"] + i * N: R["x"] + (i + 1) * N, :] = x[core * nb + i]

    Wq_eff = W_q * ln_g[:, None]                        # [1024, 1024]
    Wkv_eff = W_kv * ln_g[:, None]                      # [1024, 128]
    bq = ln_b @ W_q                                     # [1024]
    bkv = ln_b @ W_kv                                   # [128]
    Wk, Wv = Wkv_eff[:, 0:DH], Wkv_eff[:, DH:2 * DH]
    bk, bv = bkv[0:DH], bkv[DH:2 * DH]

    blob[R["wq"]:R["wq"] + DIM, :] = Wq_eff
    blob[R["wkv"]:R["wkv"] + DIM, 0:DH] = Wk
    blob[R["wkv"]:R["wkv"] + DIM, DH:2 * DH] = Wk
    blob[R["wkv"]:R["wkv"] + DIM, 128:192] = Wv
    blob[R["wout"]:R["wout"] + DIM, :] = W_out

    f = np.asarray(freqs, np.float64)                   # [1024, 32]
    blob[R["cos"]:R["cos"] + P, :] = np.tile(np.cos(f).T, (4, 1))
    s = np.sin(f).T                                     # [32, 1024]
    sm = s.copy()
    sm[0:ROT // 2, :] = -s[0:ROT // 2, :]
    blob[R["sinm"]:R["sinm"] + P, :] = np.tile(sm, (4, 1))

    tri = np.zeros((P, 5 * 512), F32)
    pidx = np.arange(P)[:, None]
    il = np.arange(512)[None, :]
    for k in range(4):
        tri[:, k * 512:(k + 1) * 512] = np.where(il >= 128 * k + pidx,
                                                 0.0, NEG)
    tri[NN:, 4 * 512:5 * 512] = NEG
    blob[R["tri"]:R["tri"] + P, :] = tri[:, 0:1024]
    blob[R["tri"] + P:R["tri"] + 2 * P, :] = tri[:, 1024:2048]
    blob[R["tri"] + 2 * P:R["tri"] + 3 * P, 0:512] = tri[:, 2048:2560]

    nk = np.asarray(null_kv[0]).T                       # [64, 2]
    blob[R["misc"]:R["misc"] + DH, MC_KTAIL:MC_KTAIL + NN] = nk
    blob[R["misc"] + DH:R["misc"] + P, MC_KTAIL:MC_KTAIL + NN] = nk
    blob[R["misc"]:R["misc"] + P,
         MC_IDENT:MC_IDENT + P] = np.eye(P, dtype=F32)
    blob[R["misc"]:R["misc"] + NN, MC_VTAIL:MC_VTAIL + DH] = \
        np.asarray(null_kv[1])
    blob[R["misc"]:R["misc"] + NN, MC_VTAIL + DH:MC_VTAIL + DH + NN] = 1.0
    for p in range(PC):
        blob[R["misc"]:R["misc"] + P, MC_QB + p] = bq[p * 128:(p + 1) * 128]
    blob[R["misc"]:R["misc"] + P, MC_KB] = np.concatenate([bk, bk])
    blob[R["misc"]:R["misc"] + DH, MC_VB] = bv
    blob[R["misc"] + NN:R["misc"] + P, MC_TAILB] = NEG

    if not mask_trivial:
        for i in range(nb):
            mrow = np.where(np.asarray(mask[core * nb + i]), 0.0, NEG)
            mb = np.zeros((P, NT * 512), F32)
            for jj in range(NT):
                mb[:, jj * 512:(jj + 1) * 512] = \
                    mrow[jj * P:(jj + 1) * P][:, None]
            for band in range(4):
                blob[R["mb"] + (i * 4 + band) * P:
                     R["mb"] + (i * 4 + band + 1) * P, :] = \
                    mb[:, band * 1024:(band + 1) * 1024]

    return {"blob": blob}


def _run(x, mask, freqs, ln_g, ln_b, W_q, W_kv, W_out, null_kv, **spmd_kwargs):
    x = np.asarray(x, F32)
    mask = np.asarray(mask)
    freqs = np.asarray(freqs, F32)
    ln_g = np.asarray(ln_g, np.float64)
    ln_b = np.asarray(ln_b, np.float64)
    W_q = np.asarray(W_q, np.float64)
    W_kv = np.asarray(W_kv, np.float64)
    W_out = np.asarray(W_out, np.float64)
    null_kv = np.asarray(null_kv, F32)

    mask_trivial = bool(mask.all())
    nc = _get_program(mask_trivial)
    in_maps = [
        _host_prep(c, x, mask, freqs, ln_g, ln_b, W_q, W_kv, W_out, null_kv,
                   mask_trivial)
        for c in range(NCORES)
    ]
    res = bass_utils.run_bass_kernel_spmd(nc, in_maps, list(range(NCORES)),
                                          **spmd_kwargs)
    out = np.empty((B, N, DIM), F32)
    for b in range(B):
        out[b] = res.results[b // NB]["out"][(b % NB) * N:(b % NB + 1) * N]
    return out, res


def kernel(x, mask, freqs, ln_g, ln_b, W_q, W_kv, W_out, null_kv):
    out, _ = _run(x, mask, freqs, ln_g, ln_b, W_q, W_kv, W_out, null_kv)
    return out


# revision 65
# speedup vs baseline: 1.1469x; 1.0050x over previous
"""Trainium2 Bass kernel for nn_Attention (LN -> QKV proj -> partial RoPE ->
null-KV prepend -> causal MQA attention -> out proj).

Dispatch-cost-aware sharding: the axon PJRT path has ~80ms pipeline-fill
latency per burst and ~1.3-1.6ms/call steady-state, growing with cores
used and buffer count while payload bytes are nearly free. So: ONE core
computing all NB=4 batches (all 16 heads), with ONE packed bf16 input
blob + ONE bf16 output tensor. Output is a disjoint batch stack (no
host reduction). Null-tail row masking rides the softmax Exp's bias AP;
causal tri masking uses K=128 identity matmuls into sim PSUM.

RoPE is applied post-projection via stream_shuffle partition rotation of
the biased q/k/v rows (rot contribution = shuffle * signed-sin + q * cos),
so no separate rot-weight projections are needed.

All compute ops keep uniform start-partitions (walrus checkSBSameStartPartition):
- k is projected twice (rows 0:64 and 64:128) so odd heads' QK matmuls run with
  lhsT/rhs both at base 64.
- rope groups live at rows base+(0:32) for base in {0, 64}; shuffles and
  combines stay within one base.
"""

import sys

for _p in ("/opt/trn_rl_repo",):
    if _p not in sys.path:
        sys.path.insert(0, _p)

import numpy as np
import ml_dtypes

import concourse.bass as bass
import concourse.tile as tile
from concourse import bacc, mybir
from concourse import bass_utils

F32 = np.float32
BF16 = ml_dtypes.bfloat16

B, N, DIM = 4, 1024, 1024
HEADS, DH = 16, 64
PC = HEADS // 2             # 8 head-pair groups, all on one core
ROT = 32
NN = 2                      # null kv
EPS = 1e-5
P = 128
NEG = -1.0e38
SCALE = DH ** -0.5
NT = N // P                 # 8 i-tiles / D-chunks
IB = N // 512               # 2 i-blocks

NB = 4                      # batches per core
NCORES = B // NB

dt = mybir.dt

ROT_SHUF = list(range(16, 32)) + list(range(0, 16))


def _chunks_for_block(b0):
    """j-tile chunks per i-block: lists of seq j-tile indices; 'T' = tail."""
    if b0 == 0:
        return [[0, 1], [2, 3], ["T"]]
    return [[0, 1], [2, 3], [4, 5], [6, 7], ["T"]]


def _prime_act_tables(arch):
    """Make Exp/Ln resolve to the single set containing both, so the
    act-table insertion pass emits one load instead of thrashing."""
    import concourse.hw_specs as hw_specs
    AF = mybir.ActivationFunctionType
    tables = hw_specs.get_activation_tables(arch)
    if "natural_log_exp_and_others" in tables:
        for name, fns in tables.items():
            if name != "natural_log_exp_and_others":
                for f in (AF.Exp, AF.Ln, AF.Square, AF.Identity, AF.Copy):
                    fns.discard(f)


def _blob_rows(nb, mask_trivial):
    """Row offsets of each section in the packed [R, 1024] bf16 blob."""
    off = {}
    r = 0
    off["x"] = r; r += nb * N
    off["wq"] = r; r += DIM          # [1024, 1024]
    off["wkv"] = r; r += DIM         # cols 0:128 = [Wk|Wk], 128:192 = Wv
    off["wout"] = r; r += DIM        # [1024, 1024]
    off["cos"] = r; r += P           # [128, 1024]
    off["sinm"] = r; r += P          # signed sin, [128, 1024]
    off["tri"] = r; r += 3 * P       # [128,2560] as 3 bands (1024,1024,512)
    off["misc"] = r; r += P          # ktail|ident|vtail|qb|kb|vb
    if not mask_trivial:
        off["mb"] = r; r += nb * 4 * P   # per-batch [128, 4096] as 4 bands
    off["_total"] = r
    return off


# misc band column layout
MC_KTAIL = 0          # [128, 128]
MC_IDENT = 128        # [128, 128]
MC_VTAIL = 256        # [128, 66]
MC_QB = 322           # [128, 8] f32
MC_KB = 330           # [128, 1] f32
MC_VB = 331           # [64, 1] f32
MC_TAILB = 332        # [128, 1] f32: 0 rows<NN else NEG (null-tail row mask)


def _build_program(nb, mask_trivial):
    nc = bacc.Bacc("TRN2", target_bir_lowering=False, debug=False)
    _prime_act_tables(nc.m.arch)

    f32, bf16 = dt.float32, dt.bfloat16
    AF = mybir.ActivationFunctionType
    OP = mybir.AluOpType

    R = _blob_rows(nb, mask_trivial)
    d_blob = nc.dram_tensor("blob", [R["_total"], 1024], bf16,
                            kind="ExternalInput")
    d_out = nc.dram_tensor("out", [nb * N, DIM], bf16,
                           kind="ExternalOutput")

    def bap(key, r0, r1, c0, c1):
        return d_blob.ap()[R[key] + r0: R[key] + r1, c0:c1]

    with tile.TileContext(nc) as tc:
        from contextlib import ExitStack

        ctx = ExitStack()
        with ctx:
            consts = ctx.enter_context(tc.tile_pool(name="consts", bufs=1))
            persist = ctx.enter_context(tc.tile_pool(name="persist", bufs=1))

            # ---- persistent SBUF tensors ----
            wq_sb = consts.tile([P, NT * 1024], bf16)      # 8 chunks x [128,1024]
            wkk_sb = consts.tile([P, NT * 128], bf16)
            wv_sb = consts.tile([P, NT * 64], bf16)
            wout_sb = consts.tile([P, PC * DIM], bf16)     # 8 pair chunks
            cos_sb = consts.tile([P, N], bf16)
            sinm_sb = consts.tile([P, N], bf16)
            tri_sb = consts.tile([P, 5 * 512], bf16)
            ktail_sb = consts.tile([P, P], bf16)
            vtail_sb = consts.tile([P, DH + 2], bf16)
            ident = consts.tile([P, P], bf16)
            qb_sb = consts.tile([P, PC], f32)
            kb_sb = consts.tile([P, 1], f32)
            vb_sb = consts.tile([DH, 1], f32)
            tailb_sb = consts.tile([P, 1], f32)
            mb_sb = None
            if not mask_trivial:
                mb_sb = persist.tile([P, NT * 512], bf16)

            qp = persist.tile([P, PC * N], bf16)           # q pairs [128, i]
            kT = persist.tile([P, N], bf16)                # k duplicated rows
            vT = persist.tile([DH, N], bf16)
            vext = persist.tile([P, 9 * (DH + 2)], bf16)   # v + dual ones cols
            ao = persist.tile([P, PC * N], bf16)           # attn out pairs

            # ---- load weights (bf16 blob -> SBUF, direct DMA) ----
            with tc.tile_pool(name="wstg", bufs=2) as stg:
                for c in range(NT):
                    nc.sync.dma_start(wq_sb[:, c * 1024:(c + 1) * 1024],
                                      bap("wq", c * P, (c + 1) * P, 0, 1024))
                    nc.sync.dma_start(wkk_sb[:, c * 128:(c + 1) * 128],
                                      bap("wkv", c * P, (c + 1) * P, 0, 128))
                    nc.sync.dma_start(wv_sb[:, c * 64:(c + 1) * 64],
                                      bap("wkv", c * P, (c + 1) * P, 128, 192))
                for p in range(PC):
                    nc.sync.dma_start(wout_sb[:, p * DIM:(p + 1) * DIM],
                                      bap("wout", p * P, (p + 1) * P, 0, 1024))
                nc.sync.dma_start(cos_sb[:], bap("cos", 0, P, 0, 1024))
                nc.sync.dma_start(sinm_sb[:], bap("sinm", 0, P, 0, 1024))
                nc.sync.dma_start(tri_sb[:, 0:1024], bap("tri", 0, P, 0, 1024))
                nc.sync.dma_start(tri_sb[:, 1024:2048],
                                  bap("tri", P, 2 * P, 0, 1024))
                nc.sync.dma_start(tri_sb[:, 2048:2560],
                                  bap("tri", 2 * P, 3 * P, 0, 512))
                nc.sync.dma_start(ktail_sb[:],
                                  bap("misc", 0, P, MC_KTAIL, MC_KTAIL + P))
                nc.sync.dma_start(ident[:],
                                  bap("misc", 0, P, MC_IDENT, MC_IDENT + P))
                nc.sync.dma_start(vtail_sb[:],
                                  bap("misc", 0, P, MC_VTAIL, MC_VTAIL + DH + 2))
                bst = stg.tile([P, 16], bf16, tag="bst", name="bst")
                nc.sync.dma_start(bst[:, 0:PC],
                                  bap("misc", 0, P, MC_QB, MC_QB + PC))
                nc.sync.dma_start(bst[:, PC:PC + 1],
                                  bap("misc", 0, P, MC_KB, MC_KB + 1))
                nc.sync.dma_start(bst[0:DH, PC + 1:PC + 2],
                                  bap("misc", 0, DH, MC_VB, MC_VB + 1))
                nc.sync.dma_start(bst[:, PC + 2:PC + 3],
                                  bap("misc", 0, P, MC_TAILB, MC_TAILB + 1))
                nc.vector.tensor_copy(qb_sb[:], bst[:, 0:PC])
                nc.vector.tensor_copy(kb_sb[:], bst[:, PC:PC + 1])
                nc.vector.tensor_copy(vb_sb[:], bst[0:DH, PC + 1:PC + 2])
                nc.vector.tensor_copy(tailb_sb[:], bst[:, PC + 2:PC + 3])

            # ---- helpers (same structure as 8-head version, PC=8) ----
            def ln_reduce_tile(ph1, t, xt, rsums, accs):
                c4 = t % 4
                nc.vector.tensor_reduce(rsums[:, c4:c4 + 1], xt[:],
                                        axis=mybir.AxisListType.X, op=OP.add)
                sq = ph1.tile([P, DIM], bf16, tag="sq", name="sq")
                nc.scalar.activation(sq[:], xt[:], AF.Square,
                                     accum_out=accs[:, c4:c4 + 1])

            def ln_stats_batch(stp, rsums, accs):
                mean = stp.tile([P, 4], f32, tag="stb", name="mean")
                nc.vector.tensor_scalar(out=mean[:], in0=rsums[:],
                                        scalar1=1.0 / DIM, scalar2=None,
                                        op0=OP.mult)
                ex2 = stp.tile([P, 4], f32, tag="stb", name="ex2")
                nc.vector.tensor_scalar(out=ex2[:], in0=accs[:],
                                        scalar1=1.0 / DIM, scalar2=None,
                                        op0=OP.mult)
                var = stp.tile([P, 4], f32, tag="stb", name="var")
                nc.vector.scalar_tensor_tensor(
                    out=var[:], in0=mean[:], scalar=-1.0, in1=mean[:],
                    op0=OP.mult, op1=OP.mult)
                nc.vector.scalar_tensor_tensor(
                    out=var[:], in0=ex2[:], scalar=EPS, in1=var[:],
                    op0=OP.add, op1=OP.add)
                nc.scalar.activation(var[:], var[:], AF.Ln)
                rstd = stp.tile([P, 4], f32, tag="stb", name="rstd")
                nc.scalar.activation(rstd[:], var[:], AF.Exp, scale=-0.5)
                negmr = stp.tile([P, 4], f32, tag="stb", name="negmr")
                nc.vector.scalar_tensor_tensor(
                    out=negmr[:], in0=mean[:], scalar=-1.0, in1=rstd[:],
                    op0=OP.mult, op1=OP.mult)
                return rstd, negmr

            def ln_xn_tile(xnT, ph1, ps1, t, xt, rstd, negmr):
                c4 = t % 4
                xn = ph1.tile([P, DIM], bf16, tag="xn", name="xn")
                nc.vector.tensor_scalar(out=xn[:], in0=xt[:],
                                        scalar1=rstd[:, c4:c4 + 1],
                                        scalar2=negmr[:, c4:c4 + 1],
                                        op0=OP.mult, op1=OP.add)
                for g in range(2):
                    pst = ps1.tile([P, 512], bf16, tag="tp", name="pst")
                    for c4b in range(4):
                        c = g * 4 + c4b
                        nc.tensor.transpose(pst[:, c4b * P:(c4b + 1) * P],
                                            xn[:, c * P:(c + 1) * P], ident[:])
                    dest = xnT[:].rearrange("p (c i) -> p c i", c=NT)[
                        :, g * 4:(g + 1) * 4, t * P:(t + 1) * P]
                    src = pst[:].rearrange("p (c i) -> p c i", c=4)
                    nc.scalar.copy(dest, src)

            def mm_proj(xnT, ps2, w_sb, wwidth, col0, cols, ib, rows=P):
                ps = ps2.tile([P, 512], f32, tag="proj", name="ps")
                for c in range(NT):
                    nc.tensor.matmul(
                        ps[0:rows, :],
                        w_sb[:, c * wwidth + col0: c * wwidth + col0 + cols],
                        xnT[:, c * N + ib * 512: c * N + ib * 512 + 512],
                        start=(c == 0), stop=(c == NT - 1))
                return ps

            def rope_rows(rp, dst, base, isl_c, sin_cols):
                """dst rows base:base+32 (cols isl_c slice of width 512):
                dst = dst*cos + shuffle(dst)*sinm."""
                rsl = slice(base, base + ROT)
                tmp = rp.tile([P, 512], bf16, tag="rt", name="rt")
                nc.vector.stream_shuffle(tmp[rsl, :], dst[rsl, isl_c], ROT_SHUF)
                nc.vector.tensor_tensor(out=dst[rsl, isl_c],
                                        in0=dst[rsl, isl_c],
                                        in1=cos_sb[rsl, sin_cols], op=OP.mult)
                nc.vector.tensor_tensor(out=tmp[rsl, :], in0=tmp[rsl, :],
                                        in1=sinm_sb[rsl, sin_cols], op=OP.mult)
                nc.vector.tensor_tensor(out=dst[rsl, isl_c],
                                        in0=dst[rsl, isl_c],
                                        in1=tmp[rsl, :], op=OP.add)

            def proj_q_pair(xnT, psq, rp, p, ib):
                isl = slice(ib * 512, (ib + 1) * 512)
                csl = slice(p * N + ib * 512, p * N + ib * 512 + 512)
                ps = mm_proj(xnT, psq, wq_sb, 1024, p * P, P, ib)
                nc.scalar.add(qp[:, csl], ps[:], qb_sb[:, p:p + 1])
                for base in (0, DH):
                    rope_rows(rp, qp, base, csl, isl)

            def proj_kv_ib(xnT, ps2, vtp, rp, ib):
                isl = slice(ib * 512, (ib + 1) * 512)
                ps = mm_proj(xnT, ps2, wkk_sb, 128, 0, P, ib)
                nc.scalar.add(kT[:, isl], ps[:], kb_sb[:])
                for base in (0, DH):
                    rope_rows(rp, kT, base, isl, isl)
                ps = mm_proj(xnT, ps2, wv_sb, 64, 0, DH, ib, rows=DH)
                nc.scalar.add(vT[:, isl], ps[0:DH, :], vb_sb[:])
                rope_rows(rp, vT, 0, isl, isl)
                # v row-major + dual ones cols for this i-block's j-tiles
                for jj in range(ib * 4, ib * 4 + 4):
                    pv = vtp.tile([P, DH], bf16, tag="vt", name="pv")
                    nc.tensor.transpose(pv[:], vT[:, jj * P:(jj + 1) * P],
                                        ident[0:DH, 0:DH])
                    vbase = jj * (DH + 2)
                    nc.vector.tensor_copy(vext[:, vbase:vbase + DH], pv[:])
                    nc.vector.memset(vext[:, vbase + DH:vbase + DH + 2], 1.0)

            # ================= per-batch pipeline =================
            for b in range(nb):
                if not mask_trivial:
                    for band in range(4):
                        nc.sync.dma_start(
                            mb_sb[:, band * 1024:(band + 1) * 1024],
                            bap("mb", (b * 4 + band) * P,
                                (b * 4 + band + 1) * P, 0, 1024))

                # ---- Phases 1+2: LN + kv projections + q pair 0 ----
                xnT = persist.tile([P, NT * N], bf16, tag="xnT",
                                   name=f"xnT{b}")
                with tc.tile_pool(name="ph1sb", bufs=4) as ph1, \
                     tc.tile_pool(name="ph1st", bufs=32) as stp, \
                     tc.tile_pool(name="ph1ps", bufs=2, space="PSUM") as ps1, \
                     tc.tile_pool(name="ph2ps", bufs=5, space="PSUM") as ps2, \
                     tc.tile_pool(name="rope", bufs=4) as rp, \
                     tc.tile_pool(name="vtp", bufs=1, space="PSUM") as vtp:
                    xts = []
                    for t in range(NT):
                        xt = ph1.tile([P, DIM], bf16, tag=f"x{t % 4}",
                                      name=f"xt{t}", bufs=2)
                        nc.gpsimd.dma_start(
                            xt[:], bap("x", b * N + t * P, b * N + (t + 1) * P,
                                       0, 1024))
                        xts.append(xt)
                    for half in range(2):
                        rsums = stp.tile([P, 4], f32, tag=f"rs{half}",
                                         name=f"rsums{half}", bufs=1)
                        accs = stp.tile([P, 4], f32, tag=f"ac{half}",
                                        name=f"accs{half}", bufs=1)
                        for t in range(half * 4, half * 4 + 4):
                            ln_reduce_tile(ph1, t, xts[t], rsums, accs)
                        rstd, negmr = ln_stats_batch(stp, rsums, accs)
                        for t in range(half * 4, half * 4 + 4):
                            ln_xn_tile(xnT, ph1, ps1, t, xts[t], rstd, negmr)
                        proj_kv_ib(xnT, ps2, vtp, rp, half)
                    for ib in range(2):
                        proj_q_pair(xnT, ps2, rp, 0, ib)
                    nc.vector.tensor_copy(vext[:, 8 * (DH + 2):9 * (DH + 2)],
                                          vtail_sb[:])

                # ---- Phase 3: attention, q-proj of pair pc+1 interleaved ----
                with tc.tile_pool(name="simps", bufs=2, space="PSUM") as simps, \
                     tc.tile_pool(name="ps2q", bufs=2, space="PSUM") as ps2q, \
                     tc.tile_pool(name="rope2", bufs=4) as rp2, \
                     tc.tile_pool(name="outps", bufs=1, space="PSUM") as outps, \
                     tc.tile_pool(name="atsb", bufs=6) as atsb, \
                     tc.tile_pool(name="nrm", bufs=2) as nrm:
                    for pc in range(PC):
                        if pc + 1 < PC:
                            for ib in range(2):
                                proj_q_pair(xnT, ps2q, rp2, pc + 1, ib)
                        rsb = nrm.tile([P, N], f32, name="rsb", tag="rsb")
                        nc.vector.memset(rsb[DH:DH + ROT, :], 1.0)
                        aots = {}
                        for b0 in range(IB):
                            chunks = _chunks_for_block(b0)
                            alljj = [jj for ch in chunks for jj in ch]
                            qhs = {}
                            psos = {}
                            for e in (0, 1):
                                hb = e * DH
                                qhs[e] = qp[hb:hb + DH,
                                            pc * N + b0 * 512:
                                            pc * N + b0 * 512 + 512]
                                psos[e] = outps.tile([P, 512], f32,
                                                     name=f"pso{e}",
                                                     tag=f"outT{e}")
                            first_av = True
                            for ch in chunks:
                                w = len(ch) * 512
                                pss = {}
                                for e in (0, 1):
                                    pss[e] = simps.tile([P, 1024], f32,
                                                        name=f"pss{e}",
                                                        tag="sim")
                                for idx, jj in enumerate(ch):
                                    for e in (0, 1):
                                        hb = e * DH
                                        seg = pss[e][:, idx * 512:(idx + 1) * 512]
                                        diag = jj != "T" and jj >= 4 * b0
                                        if jj == "T":
                                            # tail row-mask rides the exp bias
                                            extra = 0 if mask_trivial else 1
                                        else:
                                            extra = ((1 if diag else 0)
                                                     + (0 if mask_trivial
                                                        else 1))
                                        if jj == "T":
                                            nc.tensor.matmul(
                                                seg, ktail_sb[hb:hb + DH, :],
                                                qhs[e], start=True,
                                                stop=(extra == 0))
                                        else:
                                            nc.tensor.matmul(
                                                seg,
                                                kT[hb:hb + DH,
                                                   jj * P:(jj + 1) * P],
                                                qhs[e], start=True,
                                                stop=(extra == 0))
                                for idx, jj in enumerate(ch):
                                    for e in (0, 1):
                                        seg = pss[e][:, idx * 512:(idx + 1) * 512]
                                        if jj == "T":
                                            if not mask_trivial:
                                                nc.tensor.matmul(
                                                    seg, ident[:],
                                                    tri_sb[:, 4 * 512:5 * 512],
                                                    start=False, stop=True)
                                            continue
                                        diag = jj >= 4 * b0
                                        extra = ((1 if diag else 0)
                                                 + (0 if mask_trivial else 1))
                                        if diag:
                                            k = jj - 4 * b0
                                            extra -= 1
                                            nc.tensor.matmul(
                                                seg, ident[:],
                                                tri_sb[:, k * 512:(k + 1) * 512],
                                                start=False, stop=(extra == 0))
                                        if not mask_trivial:
                                            extra -= 1
                                            nc.tensor.matmul(
                                                seg, ident[:],
                                                mb_sb[:, jj * 512:(jj + 1) * 512],
                                                start=False, stop=(extra == 0))
                                ats = {}
                                for e in (0, 1):
                                    at = atsb.tile([P, 1024], bf16,
                                                   name=f"at{e}", tag=f"at{e}")
                                    if mask_trivial and ch == ["T"]:
                                        nc.scalar.activation(at[:, 0:w],
                                                             pss[e][:, 0:w],
                                                             AF.Exp, scale=SCALE,
                                                             bias=tailb_sb[:])
                                    else:
                                        nc.scalar.activation(at[:, 0:w],
                                                             pss[e][:, 0:w],
                                                             AF.Exp, scale=SCALE)
                                    ats[e] = at
                                for idx, jj in enumerate(ch):
                                    vjj = 8 if jj == "T" else jj
                                    vcols = vext[:, vjj * (DH + 2):
                                                 (vjj + 1) * (DH + 2)]
                                    for e in (0, 1):
                                        nc.tensor.matmul(
                                            psos[e][0:DH + 2, :], vcols,
                                            ats[e][:, idx * 512:(idx + 1) * 512],
                                            start=first_av,
                                            stop=(jj == alljj[-1]))
                                    first_av = False
                            bsl0 = slice(b0 * 512, (b0 + 1) * 512)
                            for e in (1, 0):
                                aot = nrm.tile([DH + 2, 512], f32,
                                               name=f"aot{b0}{e}",
                                               tag=f"aot{b0}{e}")
                                nc.vector.tensor_copy(aot[:],
                                                      psos[e][0:DH + 2, :])
                                if e == 1:
                                    nc.vector.tensor_copy(rsb[DH:DH + 2, bsl0],
                                                          aot[DH:DH + 2, :])
                                else:
                                    nc.vector.tensor_copy(rsb[DH:DH + 1, bsl0],
                                                          aot[DH:DH + 1, :])
                                aots[(b0, e)] = aot
                        rows2 = rsb[DH:DH + 2, :]
                        nc.scalar.activation(rows2, rows2, AF.Ln)
                        nc.scalar.activation(rows2, rows2, AF.Exp, scale=-1.0)
                        for e in (0, 1):
                            bc = nrm.tile([P, N], f32, name=f"bc{e}",
                                          tag=f"bc{e}")
                            nc.vector.stream_shuffle(bc[DH:DH + ROT, :],
                                                     rsb[DH:DH + ROT, :],
                                                     [e] * 32)
                            nc.sync.dma_start(bc[0:ROT, :], bc[DH:DH + ROT, :])
                            nc.sync.dma_start(bc[ROT:DH, :], bc[0:ROT, :])
                            for b0 in range(IB):
                                osl = slice(pc * N + b0 * 512,
                                            pc * N + b0 * 512 + 512)
                                bsl = slice(b0 * 512, (b0 + 1) * 512)
                                src = aots[(b0, e)]
                                if e == 0:
                                    nc.gpsimd.tensor_tensor(
                                        out=ao[0:DH, osl], in0=src[0:DH, :],
                                        in1=bc[0:DH, bsl], op=OP.mult)
                                else:
                                    tmp = nrm.tile([DH, 512], bf16,
                                                   name="tmpn", tag="tmpn")
                                    nc.gpsimd.tensor_tensor(
                                        out=tmp[:], in0=src[0:DH, :],
                                        in1=bc[0:DH, bsl], op=OP.mult)
                                    nc.sync.dma_start(ao[DH:P, osl], tmp[:])

                # ---- Phase 4: out projection ----
                with tc.tile_pool(name="opps", bufs=4, space="PSUM") as opps, \
                     tc.tile_pool(name="opsb", bufs=3) as opsb:
                    for t in range(NT):
                        orow = opsb.tile([P, DIM], bf16, tag="orow")
                        for nb2 in range(2):
                            ps = opps.tile([P, 512], f32, tag="op")
                            for p in range(PC):
                                nc.tensor.matmul(
                                    ps[:],
                                    ao[:, p * N + t * P: p * N + t * P + 128],
                                    wout_sb[:, p * DIM + nb2 * 512:
                                            p * DIM + nb2 * 512 + 512],
                                    start=(p == 0), stop=(p == PC - 1))
                            nc.scalar.copy(
                                orow[:, nb2 * 512:(nb2 + 1) * 512], ps[:])
                        nc.sync.dma_start(
                            d_out.ap()[b * N + t * P: b * N + (t + 1) * P, :],
                            orow[:])

    nc.compile()
    return nc


_PROG_CACHE = {}


def _get_program(mask_trivial, nb=NB):
    key = (nb, bool(mask_trivial))
    if key not in _PROG_CACHE:
        _PROG_CACHE[key] = _build_program(nb, key[1])
    return _PROG_CACHE[key]


def _host_prep(core, x, mask, freqs, ln_g, ln_b, W_q, W_kv, W_out, null_kv,
               mask_trivial, nb=NB):
    R = _blob_rows(nb, mask_trivial)
    blob = np.zeros((R["_total"], 1024), BF16)

    for i in range(nb):
        blob[R["x"] + i * N: R["x"] + (i + 1) * N, :] = x[core * nb + i]

    Wq_eff = W_q * ln_g[:, None]                        # [1024, 1024]
    Wkv_eff = W_kv * ln_g[:, None]                      # [1024, 128]
    bq = ln_b @ W_q                                     # [1024]
    bkv = ln_b @ W_kv                                   # [128]
    Wk, Wv = Wkv_eff[:, 0:DH], Wkv_eff[:, DH:2 * DH]
    bk, bv = bkv[0:DH], bkv[DH:2 * DH]

    blob[R["wq"]:R["wq"] + DIM, :] = Wq_eff
    blob[R["wkv"]:R["wkv"] + DIM, 0:DH] = Wk
    blob[R["wkv"]:R["wkv"] + DIM, DH:2 * DH] = Wk
    blob[R["wkv"]:R["wkv"] + DIM, 128:192] = Wv
    blob[R["wout"]:R["wout"] + DIM, :] = W_out

    f = np.asarray(freqs, np.float64)                   # [1024, 32]
    blob[R["cos"]:R["cos"] + P, :] = np.tile(np.cos(f).T, (4, 1))
    s = np.sin(f).T                                     # [32, 1024]
    sm = s.copy()
    sm[0:ROT // 2, :] = -s[0:ROT // 2, :]
    blob[R["sinm"]:R["sinm"] + P, :] = np.tile(sm, (4, 1))

    tri = np.zeros((P, 5 * 512), F32)
    pidx = np.arange(P)[:, None]
    il = np.arange(512)[None, :]
    for k in range(4):
        tri[:, k * 512:(k + 1) * 512] = np.where(il >= 128 * k + pidx,
                                                 0.0, NEG)
    tri[NN:, 4 * 512:5 * 512] = NEG
    blob[R["tri"]:R["tri"] + P, :] = tri[:, 0:1024]
    blob[R["tri"] + P:R["tri"] + 2 * P, :] = tri[:, 1024:2048]
    blob[R["tri"] + 2 * P:R["tri"] + 3 * P, 0:512] = tri[:, 2048:2560]

    nk = np.asarray(null_kv[0]).T                       # [64, 2]
    blob[R["misc"]:R["misc"] + DH, MC_KTAIL:MC_KTAIL + NN] = nk
    blob[R["misc"] + DH:R["misc"] + P, MC_KTAIL:MC_KTAIL + NN] = nk
    blob[R["misc"]:R["misc"] + P,
         MC_IDENT:MC_IDENT + P] = np.eye(P, dtype=F32)
    blob[R["misc"]:R["misc"] + NN, MC_VTAIL:MC_VTAIL + DH] = \
        np.asarray(null_kv[1])
    blob[R["misc"]:R["misc"] + NN, MC_VTAIL + DH:MC_VTAIL + DH + NN] = 1.0
    for p in range(PC):
        blob[R["misc"]:R["misc"] + P, MC_QB + p] = bq[p * 128:(p + 1) * 128]
    blob[R["misc"]:R["misc"] + P, MC_KB] = np.concatenate([bk, bk])
    blob[R["misc"]:R["misc"] + DH, MC_VB] = bv
    blob[R["misc"] + NN:R["misc"] + P, MC_TAILB] = NEG

    if not mask_trivial:
        for i in range(nb):
            mrow = np.where(np.asarray(mask[core * nb + i]), 0.0, NEG)
            mb = np.zeros((P, NT * 512), F32)
            for jj in range(NT):
                mb[:, jj * 512:(jj + 1) * 512] = \
                    mrow[jj * P:(jj + 1) * P][:, None]
            for band in range(4):
                blob[R["mb"] + (i * 4 + band) * P:
                     R["mb"] + (i * 4 + band + 1) * P, :] = \
                    mb[:, band * 1024:(band + 1) * 1024]

    return {"blob": blob}


def _run(x, mask, freqs, ln_g, ln_b, W_q, W_kv, W_out, null_kv, **spmd_kwargs):
    x = np.asarray(x, F32)
    mask = np.asarray(mask)
    freqs = np.asarray(freqs, F32)
    ln_g = np.asarray(ln_g, np.float64)
    ln_b = np.asarray(ln_b, np.float64)
    W_q = np.asarray(W_q, np.float64)
    W_kv = np.asarray(W_kv, np.float64)
    W_out = np.asarray(W_out, np.float64)
    null_kv = np.asarray(null_kv, F32)

    mask_trivial = bool(mask.all())
    nc = _get_program(mask_trivial)
    in_maps = [
        _host_prep(c, x, mask, freqs, ln_g, ln_b, W_q, W_kv, W_out, null_kv,
                   mask_trivial)
        for c in range(NCORES)
    ]
    res = bass_utils.run_bass_kernel_spmd(nc, in_maps, list(range(NCORES)),
                                          **spmd_kwargs)
    out = np.empty((B, N, DIM), F32)
    for b in range(B):
        out[b] = res.results[b // NB]["out"][(b % NB) * N:(b % NB + 1) * N]
    return out, res


def kernel(x, mask, freqs, ln_g, ln_b, W_q, W_kv, W_out, null_kv):
    out, _ = _run(x, mask, freqs, ln_g, ln_b, W_q, W_kv, W_out, null_kv)
    return out


# revision 73
# speedup vs baseline: 1.1529x; 1.0052x over previous
"""Trainium2 Bass kernel for nn_Attention (LN -> QKV proj -> partial RoPE ->
null-KV prepend -> causal MQA attention -> out proj).

Dispatch-cost-aware sharding: the axon PJRT path has ~80ms pipeline-fill
latency per burst and ~1.3-1.6ms/call steady-state, growing with cores
used and buffer count while payload bytes are nearly free. So: ONE core
computing all NB=4 batches (all 16 heads), with ONE packed bf16 input
blob + ONE bf16 output tensor. Output is a disjoint batch stack (no
host reduction). Null-tail row masking rides the softmax Exp's bias AP;
causal tri masking uses K=128 identity matmuls into sim PSUM.

RoPE is applied post-projection via stream_shuffle partition rotation of
the biased q/k/v rows (rot contribution = shuffle * signed-sin + q * cos),
so no separate rot-weight projections are needed.

All compute ops keep uniform start-partitions (walrus checkSBSameStartPartition):
- k is projected twice (rows 0:64 and 64:128) so odd heads' QK matmuls run with
  lhsT/rhs both at base 64.
- rope groups live at rows base+(0:32) for base in {0, 64}; shuffles and
  combines stay within one base.
"""

import sys

for _p in ("/opt/trn_rl_repo",):
    if _p not in sys.path:
        sys.path.insert(0, _p)

import numpy as np
import ml_dtypes

import concourse.bass as bass
import concourse.tile as tile
from concourse import bacc, mybir
from concourse import bass_utils

F32 = np.float32
BF16 = ml_dtypes.bfloat16

B, N, DIM = 4, 1024, 1024
HEADS, DH = 16, 64
PC = HEADS // 2             # 8 head-pair groups, all on one core
ROT = 32
NN = 2                      # null kv
EPS = 1e-5
P = 128
NEG = -1.0e38
SCALE = DH ** -0.5
NT = N // P                 # 8 i-tiles / D-chunks
IB = N // 512               # 2 i-blocks

NB = 4                      # batches per core
NCORES = B // NB

dt = mybir.dt

ROT_SHUF = list(range(16, 32)) + list(range(0, 16))


def _chunks_for_block(b0):
    """j-tile chunks per i-block: lists of seq j-tile indices; 'T' = tail."""
    if b0 == 0:
        return [[0, 1], [2, 3], ["T"]]
    return [[0, 1], [2, 3], [4, 5], [6, 7], ["T"]]


def _prime_act_tables(arch):
    """Make Exp/Ln resolve to the single set containing both, so the
    act-table insertion pass emits one load instead of thrashing."""
    import concourse.hw_specs as hw_specs
    AF = mybir.ActivationFunctionType
    tables = hw_specs.get_activation_tables(arch)
    if "natural_log_exp_and_others" in tables:
        for name, fns in tables.items():
            if name != "natural_log_exp_and_others":
                for f in (AF.Exp, AF.Ln, AF.Square, AF.Identity, AF.Copy):
                    fns.discard(f)


def _blob_rows(nb, mask_trivial):
    """Row offsets of each section in the packed [R, 1024] bf16 blob."""
    off = {}
    r = 0
    off["x"] = r; r += nb * N
    off["wq"] = r; r += DIM          # [1024, 1024]
    off["wkv"] = r; r += DIM         # cols 0:128 = [Wk|Wk], 128:192 = Wv
    off["wout"] = r; r += DIM        # [1024, 1024]
    off["cos"] = r; r += P           # [128, 1024]
    off["sinm"] = r; r += P          # signed sin, [128, 1024]
    off["tri"] = r; r += 3 * P       # [128,2560] as 3 bands (1024,1024,512)
    off["misc"] = r; r += P          # ktail|ident|vtail|qb|kb|vb
    if not mask_trivial:
        off["mb"] = r; r += nb * 4 * P   # per-batch [128, 4096] as 4 bands
    off["_total"] = r
    return off


# misc band column layout
MC_KTAIL = 0          # [128, 128]
MC_IDENT = 128        # [128, 128]
MC_VTAIL = 256        # [128, 66]
MC_QB = 322           # [128, 8] f32
MC_KB = 330           # [128, 1] f32
MC_VB = 331           # [64, 1] f32
MC_TAILB = 332        # [128, 1] f32: 0 rows<NN else NEG (null-tail row mask)


def _build_program(nb, mask_trivial):
    nc = bacc.Bacc("TRN2", target_bir_lowering=False, debug=False)
    _prime_act_tables(nc.m.arch)

    f32, bf16 = dt.float32, dt.bfloat16
    AF = mybir.ActivationFunctionType
    OP = mybir.AluOpType

    R = _blob_rows(nb, mask_trivial)
    d_blob = nc.dram_tensor("blob", [R["_total"], 1024], bf16,
                            kind="ExternalInput")
    d_out = nc.dram_tensor("out", [nb * N, DIM], bf16,
                           kind="ExternalOutput")

    def bap(key, r0, r1, c0, c1):
        return d_blob.ap()[R[key] + r0: R[key] + r1, c0:c1]

    with tile.TileContext(nc) as tc:
        from contextlib import ExitStack

        ctx = ExitStack()
        with ctx:
            consts = ctx.enter_context(tc.tile_pool(name="consts", bufs=1))
            persist = ctx.enter_context(tc.tile_pool(name="persist", bufs=1))

            # ---- persistent SBUF tensors ----
            wq_sb = consts.tile([P, NT * 1024], bf16)      # 8 chunks x [128,1024]
            wkk_sb = consts.tile([P, NT * 128], bf16)
            wv_sb = consts.tile([P, NT * 64], bf16)
            wout_sb = consts.tile([P, PC * DIM], bf16)     # 8 pair chunks
            cos_sb = consts.tile([P, N], bf16)
            sinm_sb = consts.tile([P, N], bf16)
            tri_sb = consts.tile([P, 5 * 512], bf16)
            ktail_sb = consts.tile([P, P], bf16)
            vtail_sb = consts.tile([P, DH + 2], bf16)
            ident = consts.tile([P, P], bf16)
            qb_sb = consts.tile([P, PC], f32)
            kb_sb = consts.tile([P, 1], f32)
            vb_sb = consts.tile([DH, 1], f32)
            tailb_sb = consts.tile([P, 1], f32)
            mb_sb = None
            if not mask_trivial:
                mb_sb = persist.tile([P, NT * 512], bf16)

            qp = persist.tile([P, PC * N], bf16)           # q pairs [128, i]
            kT = persist.tile([P, N], bf16)                # k duplicated rows
            vT = persist.tile([DH, N], bf16)
            vext = persist.tile([P, 9 * (DH + 2)], bf16)   # v + dual ones cols
            ao = persist.tile([P, PC * N], bf16)           # attn out pairs

            # ---- load weights (bf16 blob -> SBUF, direct DMA) ----
            with tc.tile_pool(name="wstg", bufs=2) as stg:
                for c in range(NT):
                    nc.sync.dma_start(wq_sb[:, c * 1024:(c + 1) * 1024],
                                      bap("wq", c * P, (c + 1) * P, 0, 1024))
                    nc.sync.dma_start(wkk_sb[:, c * 128:(c + 1) * 128],
                                      bap("wkv", c * P, (c + 1) * P, 0, 128))
                    nc.sync.dma_start(wv_sb[:, c * 64:(c + 1) * 64],
                                      bap("wkv", c * P, (c + 1) * P, 128, 192))
                for p in range(PC):
                    nc.sync.dma_start(wout_sb[:, p * DIM:(p + 1) * DIM],
                                      bap("wout", p * P, (p + 1) * P, 0, 1024))
                nc.sync.dma_start(cos_sb[:], bap("cos", 0, P, 0, 1024))
                nc.sync.dma_start(sinm_sb[:], bap("sinm", 0, P, 0, 1024))
                nc.sync.dma_start(tri_sb[:, 0:1024], bap("tri", 0, P, 0, 1024))
                nc.sync.dma_start(tri_sb[:, 1024:2048],
                                  bap("tri", P, 2 * P, 0, 1024))
                nc.sync.dma_start(tri_sb[:, 2048:2560],
                                  bap("tri", 2 * P, 3 * P, 0, 512))
                nc.sync.dma_start(ktail_sb[:],
                                  bap("misc", 0, P, MC_KTAIL, MC_KTAIL + P))
                nc.sync.dma_start(ident[:],
                                  bap("misc", 0, P, MC_IDENT, MC_IDENT + P))
                nc.sync.dma_start(vtail_sb[:],
                                  bap("misc", 0, P, MC_VTAIL, MC_VTAIL + DH + 2))
                bst = stg.tile([P, 16], bf16, tag="bst", name="bst")
                nc.sync.dma_start(bst[:, 0:PC],
                                  bap("misc", 0, P, MC_QB, MC_QB + PC))
                nc.sync.dma_start(bst[:, PC:PC + 1],
                                  bap("misc", 0, P, MC_KB, MC_KB + 1))
                nc.sync.dma_start(bst[0:DH, PC + 1:PC + 2],
                                  bap("misc", 0, DH, MC_VB, MC_VB + 1))
                nc.sync.dma_start(bst[:, PC + 2:PC + 3],
                                  bap("misc", 0, P, MC_TAILB, MC_TAILB + 1))
                nc.vector.tensor_copy(qb_sb[:], bst[:, 0:PC])
                nc.vector.tensor_copy(kb_sb[:], bst[:, PC:PC + 1])
                nc.vector.tensor_copy(vb_sb[:], bst[0:DH, PC + 1:PC + 2])
                nc.vector.tensor_copy(tailb_sb[:], bst[:, PC + 2:PC + 3])

            # ---- helpers (same structure as 8-head version, PC=8) ----
            def ln_reduce_tile(ph1, t, xt, rsums, accs):
                c4 = t % 4
                nc.vector.tensor_reduce(rsums[:, c4:c4 + 1], xt[:],
                                        axis=mybir.AxisListType.X, op=OP.add)
                sq = ph1.tile([P, DIM], bf16, tag="sq", name="sq")
                nc.scalar.activation(sq[:], xt[:], AF.Square,
                                     accum_out=accs[:, c4:c4 + 1])

            def ln_stats_batch(stp, rsums, accs):
                mean = stp.tile([P, 4], f32, tag="stb", name="mean")
                nc.vector.tensor_scalar(out=mean[:], in0=rsums[:],
                                        scalar1=1.0 / DIM, scalar2=None,
                                        op0=OP.mult)
                ex2 = stp.tile([P, 4], f32, tag="stb", name="ex2")
                nc.vector.tensor_scalar(out=ex2[:], in0=accs[:],
                                        scalar1=1.0 / DIM, scalar2=None,
                                        op0=OP.mult)
                var = stp.tile([P, 4], f32, tag="stb", name="var")
                nc.vector.scalar_tensor_tensor(
                    out=var[:], in0=mean[:], scalar=-1.0, in1=mean[:],
                    op0=OP.mult, op1=OP.mult)
                nc.vector.scalar_tensor_tensor(
                    out=var[:], in0=ex2[:], scalar=EPS, in1=var[:],
                    op0=OP.add, op1=OP.add)
                nc.scalar.activation(var[:], var[:], AF.Ln)
                rstd = stp.tile([P, 4], f32, tag="stb", name="rstd")
                nc.scalar.activation(rstd[:], var[:], AF.Exp, scale=-0.5)
                negmr = stp.tile([P, 4], f32, tag="stb", name="negmr")
                nc.vector.scalar_tensor_tensor(
                    out=negmr[:], in0=mean[:], scalar=-1.0, in1=rstd[:],
                    op0=OP.mult, op1=OP.mult)
                return rstd, negmr

            def ln_xn_tile(xnT, ph1, ps1, t, xt, rstd, negmr):
                c4 = t % 4
                xn = ph1.tile([P, DIM], bf16, tag="xn", name="xn")
                nc.vector.tensor_scalar(out=xn[:], in0=xt[:],
                                        scalar1=rstd[:, c4:c4 + 1],
                                        scalar2=negmr[:, c4:c4 + 1],
                                        op0=OP.mult, op1=OP.add)
                for g in range(2):
                    pst = ps1.tile([P, 512], bf16, tag="tp", name="pst")
                    for c4b in range(4):
                        c = g * 4 + c4b
                        nc.tensor.transpose(pst[:, c4b * P:(c4b + 1) * P],
                                            xn[:, c * P:(c + 1) * P], ident[:])
                    dest = xnT[:].rearrange("p (c i) -> p c i", c=NT)[
                        :, g * 4:(g + 1) * 4, t * P:(t + 1) * P]
                    src = pst[:].rearrange("p (c i) -> p c i", c=4)
                    nc.scalar.copy(dest, src)

            def mm_proj(xnT, ps2, w_sb, wwidth, col0, cols, ib, rows=P):
                ps = ps2.tile([P, 512], f32, tag="proj", name="ps")
                for c in range(NT):
                    nc.tensor.matmul(
                        ps[0:rows, :],
                        w_sb[:, c * wwidth + col0: c * wwidth + col0 + cols],
                        xnT[:, c * N + ib * 512: c * N + ib * 512 + 512],
                        start=(c == 0), stop=(c == NT - 1))
                return ps

            def rope_rows(rp, dst, base, isl_c, sin_cols):
                """dst rows base:base+32 (cols isl_c slice of width 512):
                dst = dst*cos + shuffle(dst)*sinm."""
                rsl = slice(base, base + ROT)
                tmp = rp.tile([P, 512], bf16, tag="rt", name="rt")
                nc.vector.stream_shuffle(tmp[rsl, :], dst[rsl, isl_c], ROT_SHUF)
                nc.vector.tensor_tensor(out=dst[rsl, isl_c],
                                        in0=dst[rsl, isl_c],
                                        in1=cos_sb[rsl, sin_cols], op=OP.mult)
                nc.vector.tensor_tensor(out=tmp[rsl, :], in0=tmp[rsl, :],
                                        in1=sinm_sb[rsl, sin_cols], op=OP.mult)
                nc.vector.tensor_tensor(out=dst[rsl, isl_c],
                                        in0=dst[rsl, isl_c],
                                        in1=tmp[rsl, :], op=OP.add)

            def proj_q_pair(xnT, psq, rp, p, ib):
                isl = slice(ib * 512, (ib + 1) * 512)
                csl = slice(p * N + ib * 512, p * N + ib * 512 + 512)
                ps = mm_proj(xnT, psq, wq_sb, 1024, p * P, P, ib)
                nc.scalar.add(qp[:, csl], ps[:], qb_sb[:, p:p + 1])
                for base in (0, DH):
                    rope_rows(rp, qp, base, csl, isl)

            def proj_kv_ib(xnT, ps2, vtp, rp, ib):
                isl = slice(ib * 512, (ib + 1) * 512)
                ps = mm_proj(xnT, ps2, wkk_sb, 128, 0, P, ib)
                nc.scalar.add(kT[:, isl], ps[:], kb_sb[:])
                for base in (0, DH):
                    rope_rows(rp, kT, base, isl, isl)
                ps = mm_proj(xnT, ps2, wv_sb, 64, 0, DH, ib, rows=DH)
                nc.scalar.add(vT[:, isl], ps[0:DH, :], vb_sb[:])
                rope_rows(rp, vT, 0, isl, isl)
                # v row-major + dual ones cols for this i-block's j-tiles
                for jj in range(ib * 4, ib * 4 + 4):
                    pv = vtp.tile([P, DH], bf16, tag="vt", name="pv")
                    nc.tensor.transpose(pv[:], vT[:, jj * P:(jj + 1) * P],
                                        ident[0:DH, 0:DH])
                    vbase = jj * (DH + 2)
                    nc.vector.tensor_copy(vext[:, vbase:vbase + DH], pv[:])
                    nc.vector.memset(vext[:, vbase + DH:vbase + DH + 2], 1.0)

            def outproj_tile(opps, opsb, bb, t):
                orow = opsb.tile([P, DIM], bf16, tag="orow")
                for nb2 in range(2):
                    ps = opps.tile([P, 512], f32, tag="op")
                    for p in range(PC):
                        nc.tensor.matmul(
                            ps[:],
                            ao[:, p * N + t * P: p * N + t * P + 128],
                            wout_sb[:, p * DIM + nb2 * 512:
                                    p * DIM + nb2 * 512 + 512],
                            start=(p == 0), stop=(p == PC - 1))
                    nc.vector.tensor_copy(
                        orow[:, nb2 * 512:(nb2 + 1) * 512], ps[:])
                nc.sync.dma_start(
                    d_out.ap()[bb * N + t * P: bb * N + (t + 1) * P, :],
                    orow[:])

            # ================= per-batch pipeline =================
            # outproj of batch b-1 is deferred into batch b's LN phase so
            # PE stays fed while DVE/Act run the LN reductions.
            for b in range(nb):
                if not mask_trivial:
                    for band in range(4):
                        nc.sync.dma_start(
                            mb_sb[:, band * 1024:(band + 1) * 1024],
                            bap("mb", (b * 4 + band) * P,
                                (b * 4 + band + 1) * P, 0, 1024))

                # ---- Phases 1+2: LN + kv projections + q pair 0 ----
                xnT = persist.tile([P, NT * N], bf16, tag="xnT",
                                   name=f"xnT{b}")
                with tc.tile_pool(name="ph1sb", bufs=4) as ph1, \
                     tc.tile_pool(name="ph1st", bufs=32) as stp, \
                     tc.tile_pool(name="ph1ps", bufs=2, space="PSUM") as ps1, \
                     tc.tile_pool(name="ph2ps", bufs=3, space="PSUM") as ps2, \
                     tc.tile_pool(name="opps", bufs=2, space="PSUM") as opps, \
                     tc.tile_pool(name="opsb", bufs=2) as opsb, \
                     tc.tile_pool(name="rope", bufs=4) as rp, \
                     tc.tile_pool(name="vtp", bufs=1, space="PSUM") as vtp:
                    xts = []
                    for t in range(NT):
                        xt = ph1.tile([P, DIM], bf16, tag=f"x{t % 4}",
                                      name=f"xt{t}", bufs=2)
                        nc.gpsimd.dma_start(
                            xt[:], bap("x", b * N + t * P, b * N + (t + 1) * P,
                                       0, 1024))
                        xts.append(xt)
                    if b > 0:
                        for t in range(NT):
                            outproj_tile(opps, opsb, b - 1, t)
                    for half in range(2):
                        rsums = stp.tile([P, 4], f32, tag=f"rs{half}",
                                         name=f"rsums{half}", bufs=1)
                        accs = stp.tile([P, 4], f32, tag=f"ac{half}",
                                        name=f"accs{half}", bufs=1)
                        for t in range(half * 4, half * 4 + 4):
                            ln_reduce_tile(ph1, t, xts[t], rsums, accs)
                        rstd, negmr = ln_stats_batch(stp, rsums, accs)
                        for t in range(half * 4, half * 4 + 4):
                            ln_xn_tile(xnT, ph1, ps1, t, xts[t], rstd, negmr)
                        proj_kv_ib(xnT, ps2, vtp, rp, half)
                    for ib in range(2):
                        proj_q_pair(xnT, ps2, rp, 0, ib)
                    nc.vector.tensor_copy(vext[:, 8 * (DH + 2):9 * (DH + 2)],
                                          vtail_sb[:])

                # ---- Phase 3: attention, q-proj of pair pc+1 interleaved ----
                with tc.tile_pool(name="simps", bufs=2, space="PSUM") as simps, \
                     tc.tile_pool(name="ps2q", bufs=2, space="PSUM") as ps2q, \
                     tc.tile_pool(name="rope2", bufs=4) as rp2, \
                     tc.tile_pool(name="outps", bufs=1, space="PSUM") as outps, \
                     tc.tile_pool(name="atsb", bufs=6) as atsb, \
                     tc.tile_pool(name="nrm", bufs=2) as nrm:
                    for pc in range(PC):
                        if pc + 1 < PC:
                            for ib in range(2):
                                proj_q_pair(xnT, ps2q, rp2, pc + 1, ib)
                        rsb = nrm.tile([P, N], f32, name="rsb", tag="rsb")
                        nc.vector.memset(rsb[DH:DH + ROT, :], 1.0)
                        aots = {}
                        for b0 in range(IB):
                            chunks = _chunks_for_block(b0)
                            alljj = [jj for ch in chunks for jj in ch]
                            qhs = {}
                            psos = {}
                            for e in (0, 1):
                                hb = e * DH
                                qhs[e] = qp[hb:hb + DH,
                                            pc * N + b0 * 512:
                                            pc * N + b0 * 512 + 512]
                                psos[e] = outps.tile([P, 512], f32,
                                                     name=f"pso{e}",
                                                     tag=f"outT{e}")
                            first_av = True
                            for ch in chunks:
                                w = len(ch) * 512
                                pss = {}
                                for e in (0, 1):
                                    pss[e] = simps.tile([P, 1024], f32,
                                                        name=f"pss{e}",
                                                        tag="sim")
                                for idx, jj in enumerate(ch):
                                    for e in (0, 1):
                                        hb = e * DH
                                        seg = pss[e][:, idx * 512:(idx + 1) * 512]
                                        diag = jj != "T" and jj >= 4 * b0
                                        if jj == "T":
                                            # tail row-mask rides the exp bias
                                            extra = 0 if mask_trivial else 1
                                        else:
                                            extra = ((1 if diag else 0)
                                                     + (0 if mask_trivial
                                                        else 1))
                                        if jj == "T":
                                            nc.tensor.matmul(
                                                seg, ktail_sb[hb:hb + DH, :],
                                                qhs[e], start=True,
                                                stop=(extra == 0))
                                        else:
                                            nc.tensor.matmul(
                                                seg,
                                                kT[hb:hb + DH,
                                                   jj * P:(jj + 1) * P],
                                                qhs[e], start=True,
                                                stop=(extra == 0))
                                for idx, jj in enumerate(ch):
                                    for e in (0, 1):
                                        seg = pss[e][:, idx * 512:(idx + 1) * 512]
                                        if jj == "T":
                                            if not mask_trivial:
                                                nc.tensor.matmul(
                                                    seg, ident[:],
                                                    tri_sb[:, 4 * 512:5 * 512],
                                                    start=False, stop=True)
                                            continue
                                        diag = jj >= 4 * b0
                                        extra = ((1 if diag else 0)
                                                 + (0 if mask_trivial else 1))
                                        if diag:
                                            k = jj - 4 * b0
                                            extra -= 1
                                            nc.tensor.matmul(
                                                seg, ident[:],
                                                tri_sb[:, k * 512:(k + 1) * 512],
                                                start=False, stop=(extra == 0))
                                        if not mask_trivial:
                                            extra -= 1
                                            nc.tensor.matmul(
                                                seg, ident[:],
                                                mb_sb[:, jj * 512:(jj + 1) * 512],
                                                start=False, stop=(extra == 0))
                                ats = {}
                                for e in (0, 1):
                                    at = atsb.tile([P, 1024], bf16,
                                                   name=f"at{e}", tag=f"at{e}")
                                    if mask_trivial and ch == ["T"]:
                                        nc.scalar.activation(at[:, 0:w],
                                                             pss[e][:, 0:w],
                                                             AF.Exp, scale=SCALE,
                                                             bias=tailb_sb[:])
                                    else:
                                        nc.scalar.activation(at[:, 0:w],
                                                             pss[e][:, 0:w],
                                                             AF.Exp, scale=SCALE)
                                    ats[e] = at
                                for idx, jj in enumerate(ch):
                                    vjj = 8 if jj == "T" else jj
                                    vcols = vext[:, vjj * (DH + 2):
                                                 (vjj + 1) * (DH + 2)]
                                    for e in (0, 1):
                                        nc.tensor.matmul(
                                            psos[e][0:DH + 2, :], vcols,
                                            ats[e][:, idx * 512:(idx + 1) * 512],
                                            start=first_av,
                                            stop=(jj == alljj[-1]))
                                    first_av = False
                            bsl0 = slice(b0 * 512, (b0 + 1) * 512)
                            for e in (1, 0):
                                aot = nrm.tile([DH + 2, 512], f32,
                                               name=f"aot{b0}{e}",
                                               tag=f"aot{b0}{e}")
                                nc.vector.tensor_copy(aot[:],
                                                      psos[e][0:DH + 2, :])
                                if e == 1:
                                    nc.vector.tensor_copy(rsb[DH:DH + 2, bsl0],
                                                          aot[DH:DH + 2, :])
                                else:
                                    nc.vector.tensor_copy(rsb[DH:DH + 1, bsl0],
                                                          aot[DH:DH + 1, :])
                                aots[(b0, e)] = aot
                        rows2 = rsb[DH:DH + 2, :]
                        nc.scalar.activation(rows2, rows2, AF.Ln)
                        nc.scalar.activation(rows2, rows2, AF.Exp, scale=-1.0)
                        for e in (0, 1):
                            bc = nrm.tile([P, N], f32, name=f"bc{e}",
                                          tag=f"bc{e}")
                            nc.vector.stream_shuffle(bc[DH:DH + ROT, :],
                                                     rsb[DH:DH + ROT, :],
                                                     [e] * 32)
                            nc.sync.dma_start(bc[0:ROT, :], bc[DH:DH + ROT, :])
                            nc.sync.dma_start(bc[ROT:DH, :], bc[0:ROT, :])
                            for b0 in range(IB):
                                osl = slice(pc * N + b0 * 512,
                                            pc * N + b0 * 512 + 512)
                                bsl = slice(b0 * 512, (b0 + 1) * 512)
                                src = aots[(b0, e)]
                                if e == 0:
                                    nc.gpsimd.tensor_tensor(
                                        out=ao[0:DH, osl], in0=src[0:DH, :],
                                        in1=bc[0:DH, bsl], op=OP.mult)
                                else:
                                    tmp = nrm.tile([DH, 512], bf16,
                                                   name="tmpn", tag="tmpn")
                                    nc.gpsimd.tensor_tensor(
                                        out=tmp[:], in0=src[0:DH, :],
                                        in1=bc[0:DH, bsl], op=OP.mult)
                                    nc.sync.dma_start(ao[DH:P, osl], tmp[:])

            # ---- deferred out projection for the last batch ----
            with tc.tile_pool(name="opps", bufs=4, space="PSUM") as opps, \
                 tc.tile_pool(name="opsb", bufs=3) as opsb:
                for t in range(NT):
                    outproj_tile(opps, opsb, nb - 1, t)

    nc.compile()
    return nc


_PROG_CACHE = {}


def _get_program(mask_trivial, nb=NB):
    key = (nb, bool(mask_trivial))
    if key not in _PROG_CACHE:
        _PROG_CACHE[key] = _build_program(nb, key[1])
    return _PROG_CACHE[key]


def _host_prep(core, x, mask, freqs, ln_g, ln_b, W_q, W_kv, W_out, null_kv,
               mask_trivial, nb=NB):
    R = _blob_rows(nb, mask_trivial)
    blob = np.zeros((R["_total"], 1024), BF16)

    for i in range(nb):
        blob[R["x"] + i * N: R["x"] + (i + 1) * N, :] = x[core * nb + i]

    Wq_eff = W_q * ln_g[:, None]                        # [1024, 1024]
    Wkv_eff = W_kv * ln_g[:, None]                      # [1024, 128]
    bq = ln_b @ W_q                                     # [1024]
    bkv = ln_b @ W_kv                                   # [128]
    Wk, Wv = Wkv_eff[:, 0:DH], Wkv_eff[:, DH:2 * DH]
    bk, bv = bkv[0:DH], bkv[DH:2 * DH]

    blob[R["wq"]:R["wq"] + DIM, :] = Wq_eff
    blob[R["wkv"]:R["wkv"] + DIM, 0:DH] = Wk
    blob[R["wkv"]:R["wkv"] + DIM, DH:2 * DH] = Wk
    blob[R["wkv"]:R["wkv"] + DIM, 128:192] = Wv
    blob[R["wout"]:R["wout"] + DIM, :] = W_out

    f = np.asarray(freqs, np.float64)                   # [1024, 32]
    blob[R["cos"]:R["cos"] + P, :] = np.tile(np.cos(f).T, (4, 1))
    s = np.sin(f).T                                     # [32, 1024]
    sm = s.copy()
    sm[0:ROT // 2, :] = -s[0:ROT // 2, :]
    blob[R["sinm"]:R["sinm"] + P, :] = np.tile(sm, (4, 1))

    tri = np.zeros((P, 5 * 512), F32)
    pidx = np.arange(P)[:, None]
    il = np.arange(512)[None, :]
    for k in range(4):
        tri[:, k * 512:(k + 1) * 512] = np.where(il >= 128 * k + pidx,
                                                 0.0, NEG)
    tri[NN:, 4 * 512:5 * 512] = NEG
    blob[R["tri"]:R["tri"] + P, :] = tri[:, 0:1024]
    blob[R["tri"] + P:R["tri"] + 2 * P, :] = tri[:, 1024:2048]
    blob[R["tri"] + 2 * P:R["tri"] + 3 * P, 0:512] = tri[:, 2048:2560]

    nk = np.asarray(null_kv[0]).T                       # [64, 2]
    blob[R["misc"]:R["misc"] + DH, MC_KTAIL:MC_KTAIL + NN] = nk
    blob[R["misc"] + DH:R["misc"] + P, MC_KTAIL:MC_KTAIL + NN] = nk
    blob[R["misc"]:R["misc"] + P,
         MC_IDENT:MC_IDENT + P] = np.eye(P, dtype=F32)
    blob[R["misc"]:R["misc"] + NN, MC_VTAIL:MC_VTAIL + DH] = \
        np.asarray(null_kv[1])
    blob[R["misc"]:R["misc"] + NN, MC_VTAIL + DH:MC_VTAIL + DH + NN] = 1.0
    for p in range(PC):
        blob[R["misc"]:R["misc"] + P, MC_QB + p] = bq[p * 128:(p + 1) * 128]
    blob[R["misc"]:R["misc"] + P, MC_KB] = np.concatenate([bk, bk])
    blob[R["misc"]:R["misc"] + DH, MC_VB] = bv
    blob[R["misc"] + NN:R["misc"] + P, MC_TAILB] = NEG

    if not mask_trivial:
        for i in range(nb):
            mrow = np.where(np.asarray(mask[core * nb + i]), 0.0, NEG)
            mb = np.zeros((P, NT * 512), F32)
            for jj in range(NT):
                mb[:, jj * 512:(jj + 1) * 512] = \
                    mrow[jj * P:(jj + 1) * P][:, None]
            for band in range(4):
                blob[R["mb"] + (i * 4 + band) * P:
                     R["mb"] + (i * 4 + band + 1) * P, :] = \
                    mb[:, band * 1024:(band + 1) * 1024]

    return {"blob": blob}


def _run(x, mask, freqs, ln_g, ln_b, W_q, W_kv, W_out, null_kv, **spmd_kwargs):
    x = np.asarray(x, F32)
    mask = np.asarray(mask)
    freqs = np.asarray(freqs, F32)
    ln_g = np.asarray(ln_g, np.float64)
    ln_b = np.asarray(ln_b, np.float64)
    W_q = np.asarray(W_q, np.float64)
    W_kv = np.asarray(W_kv, np.float64)
    W_out = np.asarray(W_out, np.float64)
    null_kv = np.asarray(null_kv, F32)

    mask_trivial = bool(mask.all())
    nc = _get_program(mask_trivial)
    in_maps = [
        _host_prep(c, x, mask, freqs, ln_g, ln_b, W_q, W_kv, W_out, null_kv,
                   mask_trivial)
        for c in range(NCORES)
    ]
    res = bass_utils.run_bass_kernel_spmd(nc, in_maps, list(range(NCORES)),
                                          **spmd_kwargs)
    out = np.empty((B, N, DIM), F32)
    for b in range(B):
        out[b] = res.results[b // NB]["out"][(b % NB) * N:(b % NB + 1) * N]
    return out, res


def kernel(x, mask, freqs, ln_g, ln_b, W_q, W_kv, W_out, null_kv):
    out, _ = _run(x, mask, freqs, ln_g, ln_b, W_q, W_kv, W_out, null_kv)
    return out


# revision 75
# speedup vs baseline: 1.1889x; 1.0312x over previous
"""Trainium2 Bass kernel for nn_Attention (LN -> QKV proj -> partial RoPE ->
null-KV prepend -> causal MQA attention -> out proj).

Dispatch-cost-aware sharding: the axon PJRT path has ~80ms pipeline-fill
latency per burst and ~1.3-1.6ms/call steady-state, growing with cores
used and buffer count while payload bytes are nearly free. So: ONE core
computing all NB=4 batches (all 16 heads), with ONE packed bf16 input
blob + ONE bf16 output tensor. Output is a disjoint batch stack (no
host reduction). Null-tail row masking rides the softmax Exp's bias AP;
causal tri masking uses K=128 identity matmuls into sim PSUM.

RoPE is applied post-projection via stream_shuffle partition rotation of
the biased q/k/v rows (rot contribution = shuffle * signed-sin + q * cos),
so no separate rot-weight projections are needed.

All compute ops keep uniform start-partitions (walrus checkSBSameStartPartition):
- k is projected twice (rows 0:64 and 64:128) so odd heads' QK matmuls run with
  lhsT/rhs both at base 64.
- rope groups live at rows base+(0:32) for base in {0, 64}; shuffles and
  combines stay within one base.
"""

import sys

for _p in ("/opt/trn_rl_repo",):
    if _p not in sys.path:
        sys.path.insert(0, _p)

import numpy as np
import ml_dtypes

import concourse.bass as bass
import concourse.tile as tile
from concourse import bacc, mybir
from concourse import bass_utils

F32 = np.float32
BF16 = ml_dtypes.bfloat16

B, N, DIM = 4, 1024, 1024
HEADS, DH = 16, 64
PC = HEADS // 2             # 8 head-pair groups, all on one core
ROT = 32
NN = 2                      # null kv
EPS = 1e-5
P = 128
NEG = -1.0e38
SCALE = DH ** -0.5
NT = N // P                 # 8 i-tiles / D-chunks
IB = N // 512               # 2 i-blocks

NB = 4                      # batches per core
NCORES = B // NB

dt = mybir.dt

ROT_SHUF = list(range(16, 32)) + list(range(0, 16))


def _chunks_for_block(b0):
    """j-tile chunks per i-block: lists of seq j-tile indices; 'T' = tail."""
    if b0 == 0:
        return [[0, 1], [2, 3], ["T"]]
    return [[0, 1], [2, 3], [4, 5], [6, 7], ["T"]]


def _prime_act_tables(arch):
    """Make Exp/Ln resolve to the single set containing both, so the
    act-table insertion pass emits one load instead of thrashing."""
    import concourse.hw_specs as hw_specs
    AF = mybir.ActivationFunctionType
    tables = hw_specs.get_activation_tables(arch)
    if "natural_log_exp_and_others" in tables:
        for name, fns in tables.items():
            if name != "natural_log_exp_and_others":
                for f in (AF.Exp, AF.Ln, AF.Square, AF.Identity, AF.Copy):
                    fns.discard(f)


def _blob_rows(nb, mask_trivial):
    """Row offsets of each section in the packed [R, 1024] bf16 blob."""
    off = {}
    r = 0
    off["x"] = r; r += nb * N
    off["wq"] = r; r += DIM          # [1024, 1024]
    off["wkv"] = r; r += DIM         # cols 0:128 = [Wk|Wk], 128:192 = Wv
    off["wout"] = r; r += DIM        # [1024, 1024]
    off["cos"] = r; r += P           # [128, 1024]
    off["sinm"] = r; r += P          # signed sin, [128, 1024]
    off["tri"] = r; r += 3 * P       # [128,2560] as 3 bands (1024,1024,512)
    off["misc"] = r; r += P          # ktail|ident|vtail|qb|kb|vb
    if not mask_trivial:
        off["mb"] = r; r += nb * 4 * P   # per-batch [128, 4096] as 4 bands
    off["_total"] = r
    return off


# misc band column layout
MC_KTAIL = 0          # [128, 128]
MC_IDENT = 128        # [128, 128]
MC_VTAIL = 256        # [128, 66]
MC_QB = 322           # [128, 8] f32
MC_KB = 330           # [128, 1] f32
MC_VB = 331           # [64, 1] f32
MC_TAILB = 332        # [128, 1] f32: 0 rows<NN else NEG (null-tail row mask)


def _build_program(nb, mask_trivial):
    nc = bacc.Bacc("TRN2", target_bir_lowering=False, debug=False)
    _prime_act_tables(nc.m.arch)

    f32, bf16 = dt.float32, dt.bfloat16
    AF = mybir.ActivationFunctionType
    OP = mybir.AluOpType

    R = _blob_rows(nb, mask_trivial)
    d_blob = nc.dram_tensor("blob", [R["_total"], 1024], bf16,
                            kind="ExternalInput")
    d_out = nc.dram_tensor("out", [nb * N, DIM], bf16,
                           kind="ExternalOutput")

    def bap(key, r0, r1, c0, c1):
        return d_blob.ap()[R[key] + r0: R[key] + r1, c0:c1]

    with tile.TileContext(nc) as tc:
        from contextlib import ExitStack

        ctx = ExitStack()
        with ctx:
            consts = ctx.enter_context(tc.tile_pool(name="consts", bufs=1))
            persist = ctx.enter_context(tc.tile_pool(name="persist", bufs=1))

            # ---- persistent SBUF tensors ----
            wq_sb = consts.tile([P, NT * 1024], bf16)      # 8 chunks x [128,1024]
            wkk_sb = consts.tile([P, NT * 128], bf16)
            wv_sb = consts.tile([P, NT * 64], bf16)
            wout_sb = consts.tile([P, PC * DIM], bf16)     # 8 pair chunks
            cos_sb = consts.tile([P, N], bf16)
            sinm_sb = consts.tile([P, N], bf16)
            tri_sb = consts.tile([P, 5 * 512], bf16)
            ktail_sb = consts.tile([P, P], bf16)
            vtail_sb = consts.tile([P, DH + 2], bf16)
            ident = consts.tile([P, P], bf16)
            qb_sb = consts.tile([P, PC], f32)
            kb_sb = consts.tile([P, 1], f32)
            vb_sb = consts.tile([DH, 1], f32)
            tailb_sb = consts.tile([P, 1], f32)
            mb_sb = None
            if not mask_trivial:
                mb_sb = persist.tile([P, NT * 512], bf16)

            qp = persist.tile([P, PC * N], bf16)           # q pairs [128, i]
            kT = persist.tile([P, N], bf16)                # k duplicated rows
            vT = persist.tile([DH, N], bf16)
            vext = persist.tile([P, 9 * (DH + 2)], bf16)   # v + dual ones cols
            ao = persist.tile([P, PC * N], bf16)           # attn out pairs

            # ---- load weights (bf16 blob -> SBUF, direct DMA) ----
            # Emitted lazily inside batch 0's phase 1, AFTER the x-tile DMAs,
            # so x wins HBM/HWDGE service order and LN starts immediately.
            def load_weights():
              with tc.tile_pool(name="wstg", bufs=2) as stg:
                for c in range(NT):
                    nc.sync.dma_start(wq_sb[:, c * 1024:(c + 1) * 1024],
                                      bap("wq", c * P, (c + 1) * P, 0, 1024))
                    nc.sync.dma_start(wkk_sb[:, c * 128:(c + 1) * 128],
                                      bap("wkv", c * P, (c + 1) * P, 0, 128))
                    nc.sync.dma_start(wv_sb[:, c * 64:(c + 1) * 64],
                                      bap("wkv", c * P, (c + 1) * P, 128, 192))
                for p in range(PC):
                    nc.sync.dma_start(wout_sb[:, p * DIM:(p + 1) * DIM],
                                      bap("wout", p * P, (p + 1) * P, 0, 1024))
                nc.sync.dma_start(cos_sb[:], bap("cos", 0, P, 0, 1024))
                nc.sync.dma_start(sinm_sb[:], bap("sinm", 0, P, 0, 1024))
                nc.sync.dma_start(tri_sb[:, 0:1024], bap("tri", 0, P, 0, 1024))
                nc.sync.dma_start(tri_sb[:, 1024:2048],
                                  bap("tri", P, 2 * P, 0, 1024))
                nc.sync.dma_start(tri_sb[:, 2048:2560],
                                  bap("tri", 2 * P, 3 * P, 0, 512))
                nc.sync.dma_start(ktail_sb[:],
                                  bap("misc", 0, P, MC_KTAIL, MC_KTAIL + P))
                nc.sync.dma_start(ident[:],
                                  bap("misc", 0, P, MC_IDENT, MC_IDENT + P))
                nc.sync.dma_start(vtail_sb[:],
                                  bap("misc", 0, P, MC_VTAIL, MC_VTAIL + DH + 2))
                bst = stg.tile([P, 16], bf16, tag="bst", name="bst")
                nc.sync.dma_start(bst[:, 0:PC],
                                  bap("misc", 0, P, MC_QB, MC_QB + PC))
                nc.sync.dma_start(bst[:, PC:PC + 1],
                                  bap("misc", 0, P, MC_KB, MC_KB + 1))
                nc.sync.dma_start(bst[0:DH, PC + 1:PC + 2],
                                  bap("misc", 0, DH, MC_VB, MC_VB + 1))
                nc.sync.dma_start(bst[:, PC + 2:PC + 3],
                                  bap("misc", 0, P, MC_TAILB, MC_TAILB + 1))
                nc.vector.tensor_copy(qb_sb[:], bst[:, 0:PC])
                nc.vector.tensor_copy(kb_sb[:], bst[:, PC:PC + 1])
                nc.vector.tensor_copy(vb_sb[:], bst[0:DH, PC + 1:PC + 2])
                nc.vector.tensor_copy(tailb_sb[:], bst[:, PC + 2:PC + 3])

            # ---- helpers (same structure as 8-head version, PC=8) ----
            def ln_reduce_tile(ph1, t, xt, rsums, accs):
                c4 = t % 4
                nc.vector.tensor_reduce(rsums[:, c4:c4 + 1], xt[:],
                                        axis=mybir.AxisListType.X, op=OP.add)
                sq = ph1.tile([P, DIM], bf16, tag="sq", name="sq")
                nc.scalar.activation(sq[:], xt[:], AF.Square,
                                     accum_out=accs[:, c4:c4 + 1])

            def ln_stats_batch(stp, rsums, accs):
                mean = stp.tile([P, 4], f32, tag="stb", name="mean")
                nc.vector.tensor_scalar(out=mean[:], in0=rsums[:],
                                        scalar1=1.0 / DIM, scalar2=None,
                                        op0=OP.mult)
                ex2 = stp.tile([P, 4], f32, tag="stb", name="ex2")
                nc.vector.tensor_scalar(out=ex2[:], in0=accs[:],
                                        scalar1=1.0 / DIM, scalar2=None,
                                        op0=OP.mult)
                var = stp.tile([P, 4], f32, tag="stb", name="var")
                nc.vector.scalar_tensor_tensor(
                    out=var[:], in0=mean[:], scalar=-1.0, in1=mean[:],
                    op0=OP.mult, op1=OP.mult)
                nc.vector.scalar_tensor_tensor(
                    out=var[:], in0=ex2[:], scalar=EPS, in1=var[:],
                    op0=OP.add, op1=OP.add)
                nc.scalar.activation(var[:], var[:], AF.Ln)
                rstd = stp.tile([P, 4], f32, tag="stb", name="rstd")
                nc.scalar.activation(rstd[:], var[:], AF.Exp, scale=-0.5)
                negmr = stp.tile([P, 4], f32, tag="stb", name="negmr")
                nc.vector.scalar_tensor_tensor(
                    out=negmr[:], in0=mean[:], scalar=-1.0, in1=rstd[:],
                    op0=OP.mult, op1=OP.mult)
                return rstd, negmr

            def ln_xn_tile(xnT, ph1, ps1, t, xt, rstd, negmr):
                c4 = t % 4
                xn = ph1.tile([P, DIM], bf16, tag="xn", name="xn")
                nc.vector.tensor_scalar(out=xn[:], in0=xt[:],
                                        scalar1=rstd[:, c4:c4 + 1],
                                        scalar2=negmr[:, c4:c4 + 1],
                                        op0=OP.mult, op1=OP.add)
                for g in range(2):
                    pst = ps1.tile([P, 512], bf16, tag="tp", name="pst")
                    for c4b in range(4):
                        c = g * 4 + c4b
                        nc.tensor.transpose(pst[:, c4b * P:(c4b + 1) * P],
                                            xn[:, c * P:(c + 1) * P], ident[:])
                    dest = xnT[:].rearrange("p (c i) -> p c i", c=NT)[
                        :, g * 4:(g + 1) * 4, t * P:(t + 1) * P]
                    src = pst[:].rearrange("p (c i) -> p c i", c=4)
                    nc.scalar.copy(dest, src)

            def mm_proj(xnT, ps2, w_sb, wwidth, col0, cols, ib, rows=P):
                ps = ps2.tile([P, 512], f32, tag="proj", name="ps")
                for c in range(NT):
                    nc.tensor.matmul(
                        ps[0:rows, :],
                        w_sb[:, c * wwidth + col0: c * wwidth + col0 + cols],
                        xnT[:, c * N + ib * 512: c * N + ib * 512 + 512],
                        start=(c == 0), stop=(c == NT - 1))
                return ps

            def rope_rows(rp, dst, base, isl_c, sin_cols):
                """dst rows base:base+32 (cols isl_c slice of width 512):
                dst = dst*cos + shuffle(dst)*sinm."""
                rsl = slice(base, base + ROT)
                tmp = rp.tile([P, 512], bf16, tag="rt", name="rt")
                nc.vector.stream_shuffle(tmp[rsl, :], dst[rsl, isl_c], ROT_SHUF)
                nc.vector.tensor_tensor(out=dst[rsl, isl_c],
                                        in0=dst[rsl, isl_c],
                                        in1=cos_sb[rsl, sin_cols], op=OP.mult)
                nc.vector.tensor_tensor(out=tmp[rsl, :], in0=tmp[rsl, :],
                                        in1=sinm_sb[rsl, sin_cols], op=OP.mult)
                nc.vector.tensor_tensor(out=dst[rsl, isl_c],
                                        in0=dst[rsl, isl_c],
                                        in1=tmp[rsl, :], op=OP.add)

            def proj_q_pair(xnT, psq, rp, p, ib):
                isl = slice(ib * 512, (ib + 1) * 512)
                csl = slice(p * N + ib * 512, p * N + ib * 512 + 512)
                ps = mm_proj(xnT, psq, wq_sb, 1024, p * P, P, ib)
                nc.scalar.add(qp[:, csl], ps[:], qb_sb[:, p:p + 1])
                for base in (0, DH):
                    rope_rows(rp, qp, base, csl, isl)

            def proj_kv_ib(xnT, ps2, vtp, rp, ib):
                isl = slice(ib * 512, (ib + 1) * 512)
                ps = mm_proj(xnT, ps2, wkk_sb, 128, 0, P, ib)
                nc.scalar.add(kT[:, isl], ps[:], kb_sb[:])
                for base in (0, DH):
                    rope_rows(rp, kT, base, isl, isl)
                ps = mm_proj(xnT, ps2, wv_sb, 64, 0, DH, ib, rows=DH)
                nc.scalar.add(vT[:, isl], ps[0:DH, :], vb_sb[:])
                rope_rows(rp, vT, 0, isl, isl)
                # v row-major + dual ones cols for this i-block's j-tiles
                for jj in range(ib * 4, ib * 4 + 4):
                    pv = vtp.tile([P, DH], bf16, tag="vt", name="pv")
                    nc.tensor.transpose(pv[:], vT[:, jj * P:(jj + 1) * P],
                                        ident[0:DH, 0:DH])
                    vbase = jj * (DH + 2)
                    nc.vector.tensor_copy(vext[:, vbase:vbase + DH], pv[:])
                    nc.vector.memset(vext[:, vbase + DH:vbase + DH + 2], 1.0)

            def outproj_tile(opps, opsb, bb, t):
                orow = opsb.tile([P, DIM], bf16, tag="orow")
                for nb2 in range(2):
                    ps = opps.tile([P, 512], f32, tag="op")
                    for p in range(PC):
                        nc.tensor.matmul(
                            ps[:],
                            ao[:, p * N + t * P: p * N + t * P + 128],
                            wout_sb[:, p * DIM + nb2 * 512:
                                    p * DIM + nb2 * 512 + 512],
                            start=(p == 0), stop=(p == PC - 1))
                    nc.vector.tensor_copy(
                        orow[:, nb2 * 512:(nb2 + 1) * 512], ps[:])
                nc.sync.dma_start(
                    d_out.ap()[bb * N + t * P: bb * N + (t + 1) * P, :],
                    orow[:])

            # ================= per-batch pipeline =================
            # outproj of batch b-1 is deferred into batch b's LN phase so
            # PE stays fed while DVE/Act run the LN reductions.
            for b in range(nb):
                if not mask_trivial:
                    for band in range(4):
                        nc.sync.dma_start(
                            mb_sb[:, band * 1024:(band + 1) * 1024],
                            bap("mb", (b * 4 + band) * P,
                                (b * 4 + band + 1) * P, 0, 1024))

                # ---- Phases 1+2: LN + kv projections + q pair 0 ----
                xnT = persist.tile([P, NT * N], bf16, tag="xnT",
                                   name=f"xnT{b}")
                with tc.tile_pool(name="ph1sb", bufs=4) as ph1, \
                     tc.tile_pool(name="ph1st", bufs=32) as stp, \
                     tc.tile_pool(name="ph1ps", bufs=2, space="PSUM") as ps1, \
                     tc.tile_pool(name="ph2ps", bufs=3, space="PSUM") as ps2, \
                     tc.tile_pool(name="opps", bufs=2, space="PSUM") as opps, \
                     tc.tile_pool(name="opsb", bufs=2) as opsb, \
                     tc.tile_pool(name="rope", bufs=4) as rp, \
                     tc.tile_pool(name="vtp", bufs=1, space="PSUM") as vtp:
                    xts = []
                    for t in range(NT):
                        xt = ph1.tile([P, DIM], bf16, tag=f"x{t % 4}",
                                      name=f"xt{t}", bufs=2)
                        nc.gpsimd.dma_start(
                            xt[:], bap("x", b * N + t * P, b * N + (t + 1) * P,
                                       0, 1024))
                        xts.append(xt)
                    if b == 0:
                        load_weights()
                    else:
                        for t in range(NT):
                            outproj_tile(opps, opsb, b - 1, t)
                    for half in range(2):
                        rsums = stp.tile([P, 4], f32, tag=f"rs{half}",
                                         name=f"rsums{half}", bufs=1)
                        accs = stp.tile([P, 4], f32, tag=f"ac{half}",
                                        name=f"accs{half}", bufs=1)
                        for t in range(half * 4, half * 4 + 4):
                            ln_reduce_tile(ph1, t, xts[t], rsums, accs)
                        rstd, negmr = ln_stats_batch(stp, rsums, accs)
                        for t in range(half * 4, half * 4 + 4):
                            ln_xn_tile(xnT, ph1, ps1, t, xts[t], rstd, negmr)
                        proj_kv_ib(xnT, ps2, vtp, rp, half)
                    for ib in range(2):
                        proj_q_pair(xnT, ps2, rp, 0, ib)
                    nc.vector.tensor_copy(vext[:, 8 * (DH + 2):9 * (DH + 2)],
                                          vtail_sb[:])

                # ---- Phase 3: attention, q-proj of pair pc+1 interleaved ----
                with tc.tile_pool(name="simps", bufs=2, space="PSUM") as simps, \
                     tc.tile_pool(name="ps2q", bufs=2, space="PSUM") as ps2q, \
                     tc.tile_pool(name="rope2", bufs=4) as rp2, \
                     tc.tile_pool(name="outps", bufs=1, space="PSUM") as outps, \
                     tc.tile_pool(name="atsb", bufs=6) as atsb, \
                     tc.tile_pool(name="nrm", bufs=2) as nrm:
                    for pc in range(PC):
                        if pc + 1 < PC:
                            for ib in range(2):
                                proj_q_pair(xnT, ps2q, rp2, pc + 1, ib)
                        rsb = nrm.tile([P, N], f32, name="rsb", tag="rsb")
                        nc.vector.memset(rsb[DH:DH + ROT, :], 1.0)
                        aots = {}
                        for b0 in range(IB):
                            chunks = _chunks_for_block(b0)
                            alljj = [jj for ch in chunks for jj in ch]
                            qhs = {}
                            psos = {}
                            for e in (0, 1):
                                hb = e * DH
                                qhs[e] = qp[hb:hb + DH,
                                            pc * N + b0 * 512:
                                            pc * N + b0 * 512 + 512]
                                psos[e] = outps.tile([P, 512], f32,
                                                     name=f"pso{e}",
                                                     tag=f"outT{e}")
                            first_av = True
                            for ch in chunks:
                                w = len(ch) * 512
                                pss = {}
                                for e in (0, 1):
                                    pss[e] = simps.tile([P, 1024], f32,
                                                        name=f"pss{e}",
                                                        tag="sim")
                                for idx, jj in enumerate(ch):
                                    for e in (0, 1):
                                        hb = e * DH
                                        seg = pss[e][:, idx * 512:(idx + 1) * 512]
                                        diag = jj != "T" and jj >= 4 * b0
                                        if jj == "T":
                                            # tail row-mask rides the exp bias
                                            extra = 0 if mask_trivial else 1
                                        else:
                                            extra = ((1 if diag else 0)
                                                     + (0 if mask_trivial
                                                        else 1))
                                        if jj == "T":
                                            nc.tensor.matmul(
                                                seg, ktail_sb[hb:hb + DH, :],
                                                qhs[e], start=True,
                                                stop=(extra == 0))
                                        else:
                                            nc.tensor.matmul(
                                                seg,
                                                kT[hb:hb + DH,
                                                   jj * P:(jj + 1) * P],
                                                qhs[e], start=True,
                                                stop=(extra == 0))
                                for idx, jj in enumerate(ch):
                                    for e in (0, 1):
                                        seg = pss[e][:, idx * 512:(idx + 1) * 512]
                                        if jj == "T":
                                            if not mask_trivial:
                                                nc.tensor.matmul(
                                                    seg, ident[:],
                                                    tri_sb[:, 4 * 512:5 * 512],
                                                    start=False, stop=True)
                                            continue
                                        diag = jj >= 4 * b0
                                        extra = ((1 if diag else 0)
                                                 + (0 if mask_trivial else 1))
                                        if diag:
                                            k = jj - 4 * b0
                                            extra -= 1
                                            nc.tensor.matmul(
                                                seg, ident[:],
                                                tri_sb[:, k * 512:(k + 1) * 512],
                                                start=False, stop=(extra == 0))
                                        if not mask_trivial:
                                            extra -= 1
                                            nc.tensor.matmul(
                                                seg, ident[:],
                                                mb_sb[:, jj * 512:(jj + 1) * 512],
                                                start=False, stop=(extra == 0))
                                ats = {}
                                for e in (0, 1):
                                    at = atsb.tile([P, 1024], bf16,
                                                   name=f"at{e}", tag=f"at{e}")
                                    if mask_trivial and ch == ["T"]:
                                        nc.scalar.activation(at[:, 0:w],
                                                             pss[e][:, 0:w],
                                                             AF.Exp, scale=SCALE,
                                                             bias=tailb_sb[:])
                                    else:
                                        nc.scalar.activation(at[:, 0:w],
                                                             pss[e][:, 0:w],
                                                             AF.Exp, scale=SCALE)
                                    ats[e] = at
                                for idx, jj in enumerate(ch):
                                    vjj = 8 if jj == "T" else jj
                                    vcols = vext[:, vjj * (DH + 2):
                                                 (vjj + 1) * (DH + 2)]
                                    for e in (0, 1):
                                        nc.tensor.matmul(
                                            psos[e][0:DH + 2, :], vcols,
                                            ats[e][:, idx * 512:(idx + 1) * 512],
                                            start=first_av,
                                            stop=(jj == alljj[-1]))
                                    first_av = False
                            bsl0 = slice(b0 * 512, (b0 + 1) * 512)
                            for e in (1, 0):
                                aot = nrm.tile([DH + 2, 512], f32,
                                               name=f"aot{b0}{e}",
                                               tag=f"aot{b0}{e}")
                                nc.vector.tensor_copy(aot[:],
                                                      psos[e][0:DH + 2, :])
                                if e == 1:
                                    nc.vector.tensor_copy(rsb[DH:DH + 2, bsl0],
                                                          aot[DH:DH + 2, :])
                                else:
                                    nc.vector.tensor_copy(rsb[DH:DH + 1, bsl0],
                                                          aot[DH:DH + 1, :])
                                aots[(b0, e)] = aot
                        rows2 = rsb[DH:DH + 2, :]
                        nc.scalar.activation(rows2, rows2, AF.Ln)
                        nc.scalar.activation(rows2, rows2, AF.Exp, scale=-1.0)
                        for e in (0, 1):
                            bc = nrm.tile([P, N], f32, name=f"bc{e}",
                                          tag=f"bc{e}")
                            nc.vector.stream_shuffle(bc[DH:DH + ROT, :],
                                                     rsb[DH:DH + ROT, :],
                                                     [e] * 32)
                            nc.sync.dma_start(bc[0:ROT, :], bc[DH:DH + ROT, :])
                            nc.sync.dma_start(bc[ROT:DH, :], bc[0:ROT, :])
                            for b0 in range(IB):
                                osl = slice(pc * N + b0 * 512,
                                            pc * N + b0 * 512 + 512)
                                bsl = slice(b0 * 512, (b0 + 1) * 512)
                                src = aots[(b0, e)]
                                if e == 0:
                                    nc.gpsimd.tensor_tensor(
                                        out=ao[0:DH, osl], in0=src[0:DH, :],
                                        in1=bc[0:DH, bsl], op=OP.mult)
                                else:
                                    tmp = nrm.tile([DH, 512], bf16,
                                                   name="tmpn", tag="tmpn")
                                    nc.gpsimd.tensor_tensor(
                                        out=tmp[:], in0=src[0:DH, :],
                                        in1=bc[0:DH, bsl], op=OP.mult)
                                    nc.sync.dma_start(ao[DH:P, osl], tmp[:])

            # ---- deferred out projection for the last batch ----
            with tc.tile_pool(name="opps", bufs=4, space="PSUM") as opps, \
                 tc.tile_pool(name="opsb", bufs=3) as opsb:
                for t in range(NT):
                    outproj_tile(opps, opsb, nb - 1, t)

    nc.compile()
    return nc


_PROG_CACHE = {}


def _get_program(mask_trivial, nb=NB):
    key = (nb, bool(mask_trivial))
    if key not in _PROG_CACHE:
        _PROG_CACHE[key] = _build_program(nb, key[1])
    return _PROG_CACHE[key]


def _host_prep(core, x, mask, freqs, ln_g, ln_b, W_q, W_kv, W_out, null_kv,
               mask_trivial, nb=NB):
    R = _blob_rows(nb, mask_trivial)
    blob = np.zeros((R["_total"], 1024), BF16)

    for i in range(nb):
        blob[R["x"] + i * N: R["x"] + (i + 1) * N, :] = x[core * nb + i]

    Wq_eff = W_q * ln_g[:, None]                        # [1024, 1024]
    Wkv_eff = W_kv * ln_g[:, None]                      # [1024, 128]
    bq = ln_b @ W_q                                     # [1024]
    bkv = ln_b @ W_kv                                   # [128]
    Wk, Wv = Wkv_eff[:, 0:DH], Wkv_eff[:, DH:2 * DH]
    bk, bv = bkv[0:DH], bkv[DH:2 * DH]

    blob[R["wq"]:R["wq"] + DIM, :] = Wq_eff
    blob[R["wkv"]:R["wkv"] + DIM, 0:DH] = Wk
    blob[R["wkv"]:R["wkv"] + DIM, DH:2 * DH] = Wk
    blob[R["wkv"]:R["wkv"] + DIM, 128:192] = Wv
    blob[R["wout"]:R["wout"] + DIM, :] = W_out

    f = np.asarray(freqs, np.float64)                   # [1024, 32]
    blob[R["cos"]:R["cos"] + P, :] = np.tile(np.cos(f).T, (4, 1))
    s = np.sin(f).T                                     # [32, 1024]
    sm = s.copy()
    sm[0:ROT // 2, :] = -s[0:ROT // 2, :]
    blob[R["sinm"]:R["sinm"] + P, :] = np.tile(sm, (4, 1))

    tri = np.zeros((P, 5 * 512), F32)
    pidx = np.arange(P)[:, None]
    il = np.arange(512)[None, :]
    for k in range(4):
        tri[:, k * 512:(k + 1) * 512] = np.where(il >= 128 * k + pidx,
                                                 0.0, NEG)
    tri[NN:, 4 * 512:5 * 512] = NEG
    blob[R["tri"]:R["tri"] + P, :] = tri[:, 0:1024]
    blob[R["tri"] + P:R["tri"] + 2 * P, :] = tri[:, 1024:2048]
    blob[R["tri"] + 2 * P:R["tri"] + 3 * P, 0:512] = tri[:, 2048:2560]

    nk = np.asarray(null_kv[0]).T                       # [64, 2]
    blob[R["misc"]:R["misc"] + DH, MC_KTAIL:MC_KTAIL + NN] = nk
    blob[R["misc"] + DH:R["misc"] + P, MC_KTAIL:MC_KTAIL + NN] = nk
    blob[R["misc"]:R["misc"] + P,
         MC_IDENT:MC_IDENT + P] = np.eye(P, dtype=F32)
    blob[R["misc"]:R["misc"] + NN, MC_VTAIL:MC_VTAIL + DH] = \
        np.asarray(null_kv[1])
    blob[R["misc"]:R["misc"] + NN, MC_VTAIL + DH:MC_VTAIL + DH + NN] = 1.0
    for p in range(PC):
        blob[R["misc"]:R["misc"] + P, MC_QB + p] = bq[p * 128:(p + 1) * 128]
    blob[R["misc"]:R["misc"] + P, MC_KB] = np.concatenate([bk, bk])
    blob[R["misc"]:R["misc"] + DH, MC_VB] = bv
    blob[R["misc"] + NN:R["misc"] + P, MC_TAILB] = NEG

    if not mask_trivial:
        for i in range(nb):
            mrow = np.where(np.asarray(mask[core * nb + i]), 0.0, NEG)
            mb = np.zeros((P, NT * 512), F32)
            for jj in range(NT):
                mb[:, jj * 512:(jj + 1) * 512] = \
                    mrow[jj * P:(jj + 1) * P][:, None]
            for band in range(4):
                blob[R["mb"] + (i * 4 + band) * P:
                     R["mb"] + (i * 4 + band + 1) * P, :] = \
                    mb[:, band * 1024:(band + 1) * 1024]

    return {"blob": blob}


def _run(x, mask, freqs, ln_g, ln_b, W_q, W_kv, W_out, null_kv, **spmd_kwargs):
    x = np.asarray(x, F32)
    mask = np.asarray(mask)
    freqs = np.asarray(freqs, F32)
    ln_g = np.asarray(ln_g, np.float64)
    ln_b = np.asarray(ln_b, np.float64)
    W_q = np.asarray(W_q, np.float64)
    W_kv = np.asarray(W_kv, np.float64)
    W_out = np.asarray(W_out, np.float64)
    null_kv = np.asarray(null_kv, F32)

    mask_trivial = bool(mask.all())
    nc = _get_program(mask_trivial)
    in_maps = [
        _host_prep(c, x, mask, freqs, ln_g, ln_b, W_q, W_kv, W_out, null_kv,
                   mask_trivial)
        for c in range(NCORES)
    ]
    res = bass_utils.run_bass_kernel_spmd(nc, in_maps, list(range(NCORES)),
                                          **spmd_kwargs)
    out = np.empty((B, N, DIM), F32)
    for b in range(B):
        out[b] = res.results[b // NB]["out"][(b % NB) * N:(b % NB + 1) * N]
    return out, res


def kernel(x, mask, freqs, ln_g, ln_b, W_q, W_kv, W_out, null_kv):
    out, _ = _run(x, mask, freqs, ln_g, ln_b, W_q, W_kv, W_out, null_kv)
    return out
